# revision 42
# baseline (speedup 1.0000x reference)
"""GNN message-passing kernel for Trainium2, sharded over 8 NeuronCores.

Strategy (matches the "shard nodes by destination row" plan):
  * Nodes (rows of x / segment_sum outputs) are sharded across the 8 cores.
  * h = x @ W.T + b is computed shard-locally on the PE, then AllGathered so
    every core holds the full node table (bf16).
  * Each spmm is executed as, per 128-row destination tile:
      - dma_gather of the source rows x[cols] (bf16, 512B rows) from the
        all-gathered table in HBM into SBUF, edges pre-sorted by dest tile.
      - a per-batch [128 edges x 128 slots] "val-scaled one-hot" matrix built
        on the DVE in one fused tensor_scalar (is_equal -> mult) op.
      - PE matmul psum[slot, :] += onehot.T @ gathered accumulating all edge
        batches of the tile (the segment-sum).
  * Steps are separated by AllGathers of the freshly-computed state shard.
  * The last state stays in fp32 PSUM and goes through LayerNorm + exact-erf
    GELU before being written to the output shard.

All adjacency preprocessing (edge partitioning by destination, sorting,
padding to 128-edge batches, int16 index packing for dma_gather) happens on
the host in numpy inside kernel().
"""

import math
import os
import sys
from contextlib import ExitStack
from dataclasses import dataclass, field

import numpy as np

_TRN_REPO = "/opt/trn_rl_repo"
if _TRN_REPO not in sys.path and not any("trn_rl_repo" in p for p in sys.path):
    sys.path.insert(0, _TRN_REPO)

import ml_dtypes  # noqa: E402

import concourse.bass as bass  # noqa: E402
import concourse.bacc as bacc  # noqa: E402
import concourse.mybir as mybir  # noqa: E402
import concourse.tile as tile  # noqa: E402
from concourse.bass import ts  # noqa: E402
from concourse.bass_utils import run_bass_kernel_spmd  # noqa: E402

F32 = mybir.dt.float32
BF16 = mybir.dt.bfloat16
I16 = mybir.dt.int16
AF = mybir.ActivationFunctionType
ALU = mybir.AluOpType
AX = mybir.AxisListType

LN_EPS = 1e-5
P = 128  # partitions / tile rows
DMA_SCRATCH = 16384  # SWDGE descriptor carveout bytes/partition


@dataclass
class Cfg:
    n_nodes: int = 50000
    d: int = 256
    n_cores: int = 8
    n_step: int = 3
    half: int = 32768  # int16 index range for dma_gather
    # gather group sizes (dest tiles per dma_gather call) per step
    group_sizes: tuple = (6, 3, 2)
    gelu: str = "erf"  # "erf" (exact, HW), "tanh" (sim fallback)
    n_queues: int = 4  # SWDGE descriptor queues for dma_gather

    @property
    def rpc(self):  # rows per core
        return (self.n_nodes + self.n_cores - 1) // self.n_cores

    @property
    def tpc(self):  # 128-row tiles per core
        return (self.rpc + P - 1) // P

    @property
    def lp(self):  # padded local rows
        return self.tpc * P

    @property
    def ntot(self):  # padded total rows (all-gathered table size)
        return self.lp * self.n_cores


# ---------------------------------------------------------------------------
# host-side preprocessing
# ---------------------------------------------------------------------------


@dataclass
class SpmmPlan:
    step: int
    src: int  # 0 = h, 1 = state1, 2 = state2
    # per dest tile: (Blo, Bhi) batch counts (identical across cores)
    B: list = field(default_factory=list)
    TB: int = 0  # total batches = sum(Blo+Bhi)
    idx_cols: int = 0
    # per group: list of ((c0_lo, GBlo), (c0_hi, GBhi)) idx column starts
    calls: list = field(default_factory=list)
    # per tile: (bb0_lo, goff_lo, bb0_hi, goff_hi); goff = batch offset inside
    # the (group, half) gathered buffer
    tinfo: list = field(default_factory=list)


@dataclass
class Plan:
    cfg: Cfg
    spmms: list  # list[SpmmPlan]
    groups: list  # per step: list of list of dest-tile indices
    steps: list  # per step: list of spmm indices


def _pack_positions(g, cfg):
    """global node id -> (half, row) in the split half-tables.

    Each core's rows are split at hsplit = lp/2; half-0 rows of all cores
    form the "lo" table (AllGather #1), half-1 rows the "hi" table
    (AllGather #2). Both tables have n_cores*hsplit rows < 32768, so the
    int16 dma_gather indices cover them."""
    hs = cfg.lp // 2
    m = g // cfg.rpc
    r = g - m * cfg.rpc
    half = (r >= hs).astype(np.int64)
    return half, m * hs + (r - half * hs)


def make_plan_and_inputs(inputs, cfg: Cfg):
    x = np.asarray(inputs["x"], dtype=np.float32)
    adj_rows = np.asarray(inputs["adj_rows"])
    adj_cols = np.asarray(inputs["adj_cols"])
    adj_vals = np.asarray(inputs["adj_vals"], dtype=np.float32)
    idxes_seq = np.asarray(inputs["idxes_seq"]).astype(np.int64)
    idxes_res = np.asarray(inputs["idxes_res"]).astype(np.int64)
    W = np.asarray(inputs["W"], dtype=np.float32)
    b = np.asarray(inputs["b"], dtype=np.float32)
    gamma = np.asarray(inputs["gamma"], dtype=np.float32)
    beta = np.asarray(inputs["beta"], dtype=np.float32)

    nc_, d, tpc = cfg.n_cores, cfg.d, cfg.tpc

    # spmm list: (step, adj_idx, src_state)
    spmm_defs = []
    off = 0
    for i in range(cfg.n_step):
        spmm_defs.append((i, int(idxes_seq[i]), i))
        for j in range(i):
            spmm_defs.append((i, int(idxes_res[off + j]), j))
        off += i
    # order inside a step: seq first then res (matches construction order)
    steps = [[] for _ in range(cfg.n_step)]
    for k, (s, _, _) in enumerate(spmm_defs):
        steps[s].append(k)

    groups = []
    for s in range(cfg.n_step):
        gsz = cfg.group_sizes[s]
        groups.append([list(range(t0, min(t0 + gsz, tpc)))
                      for t0 in range(0, tpc, gsz)])

    # ---- bucket the edges --------------------------------------------------
    # per spmm, per core: sorted arrays + counts
    percore = []  # [k][m] -> dict(i16, rl, v, counts[t,h])
    spmms = []
    for k, (s, a, src) in enumerate(spmm_defs):
        rows = adj_rows[a].astype(np.int64)
        cols = adj_cols[a].astype(np.int64)
        vals = adj_vals[a]
        owner = rows // cfg.rpc
        half_all, ps_all = _pack_positions(cols, cfg)
        cores = []
        counts_all = np.zeros((nc_, tpc, 2), dtype=np.int64)
        for m in range(nc_):
            mask = owner == m
            lr = rows[mask] - m * cfg.rpc
            t = lr // P
            rl = (lr % P).astype(np.float32)
            h = half_all[mask]
            i16 = ps_all[mask].astype(np.int16)
            v = vals[mask]
            key = t * 2 + h
            order = np.argsort(key, kind="stable")
            key = key[order]
            cnt = np.bincount(key, minlength=tpc * 2).reshape(tpc, 2)
            counts_all[m] = cnt
            cores.append(dict(i16=i16[order], rl=rl[order], v=v[order],
                              key=key))
        cmax = counts_all.max(axis=0)  # [tpc, 2]
        B = []
        for t in range(tpc):
            blo = max(1, math.ceil(cmax[t, 0] / P))
            bhi = math.ceil(cmax[t, 1] / P)
            B.append((blo, bhi))
        sp = SpmmPlan(step=s, src=src, B=B)
        sp.TB = sum(bl + bh for bl, bh in B)
        # idx layout: per group: [lo buckets t-major][hi buckets t-major]
        # meta layout: per group: t-major: [lo batches][hi batches]
        calls = []
        c0 = 0
        for g_ts in groups[s]:
            entry = []
            for h in (0, 1):
                GB = sum(B[t][h] for t in g_ts)
                entry.append((c0, GB))
                c0 += GB * 8
            calls.append(entry)
        sp.calls = calls
        sp.idx_cols = c0
        tinfo = [None] * tpc
        bb = 0
        for g_ts in groups[s]:
            golo = 0
            gohi = 0
            # meta order within group: t asc, lo then hi
            for t in g_ts:
                bl, bh = B[t]
                tinfo[t] = (bb, golo, bb + bl, gohi)
                bb += bl + bh
                golo += bl
                gohi += bh
        sp.tinfo = tinfo
        spmms.append(sp)
        percore.append(cores)

    plan = Plan(cfg=cfg, spmms=spmms, groups=groups, steps=steps)
    plan.maxnb = max(bl + bh for sp in spmms for (bl, bh) in sp.B)

    # ---- per-core input arrays --------------------------------------------
    iota = np.broadcast_to(
        np.tile(np.arange(P, dtype=np.float32), plan.maxnb).astype(
            ml_dtypes.bfloat16), (P, plan.maxnb * P)).copy()

    # full x in split-table order: half-tables of n_cores*hsplit rows,
    # table row m*hs + r <- global node m*rpc + (half*hs + r)
    hs = cfg.lp // 2
    nt2 = hs * nc_
    x_tabs = []
    for half in range(2):
        xt = np.zeros((nt2, d), dtype=np.float32)
        for m in range(nc_):
            g0 = m * cfg.rpc + half * hs
            g1 = min(m * cfg.rpc + min((half + 1) * hs, cfg.rpc),
                     cfg.n_nodes)
            if g1 > g0:
                xt[m * hs: m * hs + (g1 - g0)] = x[g0:g1]
        x_tabs.append(np.ascontiguousarray(xt.T).astype(ml_dtypes.bfloat16))

    in_maps = []
    for m in range(nc_):
        im = {}
        im["xT_lo"] = x_tabs[0]
        im["xT_hi"] = x_tabs[1]
        im["WT"] = np.ascontiguousarray(W.T).astype(ml_dtypes.bfloat16)
        im["bias_bc"] = np.broadcast_to(b, (P, d)).copy()
        im["bias_row"] = b.reshape(1, d).astype(ml_dtypes.bfloat16)
        im["gamma_bc"] = np.broadcast_to(gamma, (P, d)).copy()
        im["beta_bc"] = np.broadcast_to(beta, (P, d)).copy()
        im["iota"] = iota

        for k, sp in enumerate(spmms):
            cd = percore[k][m]
            bounds = np.searchsorted(cd["key"], np.arange(tpc * 2 + 1))
            # --- idx array (call order: group -> half -> t) ---
            idx_chunks = []
            for gi, g_ts in enumerate(plan.groups[sp.step]):
                for h in (0, 1):
                    for t in g_ts:
                        Bn = sp.B[t][h]
                        if Bn == 0:
                            continue
                        lo_, hi_ = bounds[t * 2 + h], bounds[t * 2 + h + 1]
                        seg = cd["i16"][lo_:hi_]
                        padv = seg[-1] if len(seg) else np.int16(0)
                        pad = np.full(Bn * P - len(seg), padv, dtype=np.int16)
                        idx_chunks.append(np.concatenate([seg, pad]))
            flat = np.concatenate(idx_chunks) if idx_chunks else np.zeros(
                0, np.int16)
            cols = flat.reshape(-1, 16).T  # [16, cols]
            im[f"idx{k}"] = np.tile(cols, (8, 1)).copy()
            # --- meta arrays (order: group -> t -> lo,hi) ---
            rl_chunks = []
            v_chunks = []
            for gi, g_ts in enumerate(plan.groups[sp.step]):
                for t in g_ts:
                    for h in (0, 1):
                        Bn = sp.B[t][h]
                        if Bn == 0:
                            continue
                        lo_, hi_ = bounds[t * 2 + h], bounds[t * 2 + h + 1]
                        npad = Bn * P - (hi_ - lo_)
                        rl_chunks.append(np.concatenate(
                            [cd["rl"][lo_:hi_],
                             np.zeros(npad, np.float32)]))
                        v_chunks.append(np.concatenate(
                            [cd["v"][lo_:hi_], np.zeros(npad, np.float32)]))
            rl_flat = np.concatenate(rl_chunks)
            v_flat = np.concatenate(v_chunks)
            im[f"rloc{k}"] = np.ascontiguousarray(
                rl_flat.reshape(sp.TB, P).T).astype(ml_dtypes.bfloat16)
            im[f"vals{k}"] = np.ascontiguousarray(
                v_flat.reshape(sp.TB, P).T).astype(ml_dtypes.bfloat16)
        in_maps.append(im)

    return plan, in_maps


# ---------------------------------------------------------------------------
# device program
# ---------------------------------------------------------------------------


def _patch_lane_by_queue(n_queues):
    """Pin Tile's DMASW completion-sem lanes to SWDGE queues.

    The ucode locks each completion semaphore to one SWDGE queue; Tile's
    default round-robin over all 8 lanes ignores queue_num and mixes them.
    Give each queue a dedicated block of lanes instead."""
    from concourse import tile_sem_assignment as tsa
    if getattr(tsa.TileClockTick, "_gnn_patched", 0) == n_queues:
        return
    orig = getattr(tsa.TileClockTick, "_gnn_orig_assign_tick",
                   tsa.TileClockTick._assign_tick)

    def patched(self, inst):
        qn = getattr(inst, "queue_num", None)
        if (qn is not None and inst.engine == mybir.EngineType.Pool
                and isinstance(inst, tsa.DMAInst)):
            if not hasattr(self, "_gnn_q_rr"):
                self._gnn_q_rr = {}
            lpq = max(1, self.swdge_sem_count // n_queues)
            r = self._gnn_q_rr.get(qn, 0)
            self._gnn_q_rr[qn] = (r + 1) % lpq
            self.next_sw_dma_idx = (qn * lpq + r) % self.swdge_sem_count
        return orig(self, inst)

    tsa.TileClockTick._gnn_orig_assign_tick = orig
    tsa.TileClockTick._assign_tick = patched
    tsa.TileClockTick._gnn_patched = n_queues


def build_program(plan: Plan):
    cfg = plan.cfg
    _patch_lane_by_queue(cfg.n_queues)
    d, tpc, lp, ntot = cfg.d, cfg.tpc, cfg.lp, cfg.ntot
    nc = bacc.Bacc("TRN2", target_bir_lowering=False, debug=False,
                   num_devices=cfg.n_cores,
                   dynamic_dma_scratch_size=DMA_SCRATCH,
                   num_swdge_queues=cfg.n_queues)

    hs = lp // 2
    nt2 = hs * cfg.n_cores
    xT_lo = nc.dram_tensor("xT_lo", [d, nt2], BF16, kind="ExternalInput")
    xT_hi = nc.dram_tensor("xT_hi", [d, nt2], BF16, kind="ExternalInput")
    WT = nc.dram_tensor("WT", [d, d], BF16, kind="ExternalInput")
    bias_bc = nc.dram_tensor("bias_bc", [P, d], F32, kind="ExternalInput")
    bias_row = nc.dram_tensor("bias_row", [1, d], BF16, kind="ExternalInput")
    gamma_bc = nc.dram_tensor("gamma_bc", [P, d], F32, kind="ExternalInput")
    beta_bc = nc.dram_tensor("beta_bc", [P, d], F32, kind="ExternalInput")
    iota_d = nc.dram_tensor("iota", [P, plan.maxnb * P], BF16,
                            kind="ExternalInput")
    idx_d, rloc_d, vals_d = [], [], []
    for k, sp in enumerate(plan.spmms):
        idx_d.append(nc.dram_tensor(f"idx{k}", [P, sp.idx_cols], I16,
                                    kind="ExternalInput"))
        rloc_d.append(nc.dram_tensor(f"rloc{k}", [P, sp.TB], BF16,
                                     kind="ExternalInput"))
        vals_d.append(nc.dram_tensor(f"vals{k}", [P, sp.TB], BF16,
                                     kind="ExternalInput"))
    out_d = nc.dram_tensor("out", [lp, d], F32, kind="ExternalOutput")

    # state 0 (h) is computed in full locally; states 1..2 are computed as
    # shards and all-gathered into split half-tables (two pipelined AGs)
    shards = [None] + [nc.dram_tensor(f"s{j}_shard", [lp, d], BF16)
                       for j in range(1, cfg.n_step)]
    tabs = [(nc.dram_tensor("h_lo", [nt2, d], BF16),
             nc.dram_tensor("h_hi", [nt2, d], BF16))]
    for j in range(1, cfg.n_step):
        tabs.append((nc.dram_tensor(f"s{j}_lo", [nt2, d], BF16,
                                    addr_space="Shared"),
                     nc.dram_tensor(f"s{j}_hi", [nt2, d], BF16,
                                    addr_space="Shared")))
    RG = [list(range(cfg.n_cores))]

    with ExitStack() as ctx:
        tc = ctx.enter_context(tile.TileContext(nc, num_cores=cfg.n_cores))
        const = ctx.enter_context(tc.tile_pool(name="const", bufs=1))

        iota_sb = const.tile([P, plan.maxnb * P], BF16)
        nc.sync.dma_start(iota_sb[:], iota_d[:, :])
        bias_sb = const.tile([P, d], F32)
        nc.sync.dma_start(bias_sb[:], bias_bc[:, :])
        gamma_sb = const.tile([P, d], F32)
        nc.sync.dma_start(gamma_sb[:], gamma_bc[:, :])
        beta_sb = const.tile([P, d], F32)
        nc.sync.dma_start(beta_sb[:], beta_bc[:, :])
        eps_sb = const.tile([P, 1], F32)
        nc.vector.memset(eps_sb[:], LN_EPS)
        half_sb = const.tile([P, 1], F32)
        nc.vector.memset(half_sb[:], 0.5)

        # ---------------- phase: h = x @ W.T + b (full table, local) ------
        # bias folded into the PSUM accumulation via a K=1 ones x b matmul.
        ones_sb = const.tile([1, P], BF16)
        nc.vector.memset(ones_sb[:], 1.0)
        brow_sb = const.tile([1, d], BF16)
        nc.sync.dma_start(brow_sb[:], bias_row[:, :])
        HG = 8  # h tiles loaded per DMA
        nh2 = nt2 // P
        with tc.tile_pool(name="hph", bufs=1) as hp, \
                tc.tile_pool(name="hxt", bufs=3) as hxp, \
                tc.tile_pool(name="hpsum", bufs=8, space="PSUM") as psh, \
                tc.tile_pool(name="hout", bufs=6) as hop:
            wt0 = hp.tile([P, d], BF16, tag="wt0")
            nc.sync.dma_start(wt0[:], WT[0:P, :])
            wt1 = hp.tile([P, d], BF16, tag="wt1")
            nc.sync.dma_start(wt1[:], WT[P:2 * P, :])
            for half, (xsrc, htab) in enumerate(
                    ((xT_lo, tabs[0][0]), (xT_hi, tabs[0][1]))):
                for jg in range(0, nh2, HG):
                    ng = min(HG, nh2 - jg)
                    xt0 = hxp.tile([P, HG * P], BF16, tag="xt0")
                    nc.sync.dma_start(xt0[:, 0:ng * P],
                                      xsrc[0:P, jg * P:(jg + ng) * P])
                    xt1 = hxp.tile([P, HG * P], BF16, tag="xt1")
                    nc.sync.dma_start(xt1[:, 0:ng * P],
                                      xsrc[P:2 * P, jg * P:(jg + ng) * P])
                    for j in range(ng):
                        psum = psh.tile([P, d], F32)
                        nc.tensor.matmul(psum[:], xt0[:, ts(j, P)], wt0[:],
                                         start=True, stop=False)
                        nc.tensor.matmul(psum[:], xt1[:, ts(j, P)], wt1[:],
                                         start=False, stop=False)
                        nc.tensor.matmul(psum[:], ones_sb[:], brow_sb[:],
                                         start=False, stop=True)
                        hsb = hop.tile([P, d], BF16)
                        nc.vector.tensor_copy(hsb[:], psum[:])
                        nc.sync.dma_start(htab[ts(jg + j, P), :], hsb[:])

        # ---------------- spmm steps --------------------------------------
        for s in range(cfg.n_step):
            contribs = plan.steps[s]
            maxgb = [[1, 1] for _ in contribs]
            for ci, k in enumerate(contribs):
                for entry in plan.spmms[k].calls:
                    for h in (0, 1):
                        maxgb[ci][h] = max(maxgb[ci][h], entry[h][1])
            with ExitStack() as sctx:
                mp = sctx.enter_context(
                    tc.tile_pool(name=f"meta{s}", bufs=1))
                ip = sctx.enter_context(
                    tc.tile_pool(name=f"idxp{s}", bufs=3))
                gp = sctx.enter_context(
                    tc.tile_pool(name=f"gath{s}", bufs=2))
                vp = sctx.enter_context(
                    tc.tile_pool(name=f"vh{s}", bufs=3))
                pp = sctx.enter_context(
                    tc.tile_pool(name=f"ps{s}", bufs=8, space="PSUM"))
                op = sctx.enter_context(
                    tc.tile_pool(name=f"so{s}", bufs=3))

                rloc_sb, vals_sb = {}, {}
                maxixg = {}
                for k in contribs:
                    sp = plan.spmms[k]
                    rloc_sb[k] = mp.tile([P, sp.TB], BF16, tag=f"rl{k}",
                                         name=f"rl{k}")
                    nc.sync.dma_start(rloc_sb[k][:], rloc_d[k][:, :])
                    vals_sb[k] = mp.tile([P, sp.TB], BF16, tag=f"vl{k}",
                                         name=f"vl{k}")
                    nc.sync.dma_start(vals_sb[k][:], vals_d[k][:, :])
                    maxixg[k] = max((c[0][1] + c[1][1]) * 8
                                    for c in sp.calls)

                nreg = nc.gpsimd.alloc_register(f"nidx{s}")
                qctr = 0
                for gi, g_ts in enumerate(plan.groups[s]):
                    gt = {}
                    for ci, k in enumerate(contribs):
                        sp = plan.spmms[k]
                        (c0_lo, GBlo), (c0_hi, GBhi) = sp.calls[gi]
                        cols_g = (GBlo + GBhi) * 8
                        ixt = ip.tile([P, maxixg[k]], I16, tag=f"ixg{k}",
                                      name=f"ixg{k}")
                        nc.sync.dma_start(ixt[:, 0:cols_g],
                                          idx_d[k][:, c0_lo:c0_lo + cols_g])
                        for h, GB, cg0 in ((0, GBlo, 0), (1, GBhi, GBlo * 8)):
                            if GB == 0:
                                continue
                            g_tile = gp.tile([P, maxgb[ci][h], d], BF16,
                                             tag=f"g{k}_{h}")
                            in_ap = tabs[sp.src][h][:, :]
                            nc.gpsimd.reg_mov(nreg, GB * P)
                            nc.gpsimd.dma_gather(
                                g_tile[:, 0:GB, :], in_ap,
                                ixt[:, cg0:cg0 + GB * 8],
                                num_idxs=GB * P, num_idxs_reg=nreg,
                                elem_size=d,
                                single_packet=(GB * P <= 1024),
                                queue_num=qctr % cfg.n_queues)
                            qctr += 1
                            gt[(k, h)] = g_tile
                    for t in g_ts:
                        nmm = sum(plan.spmms[k].B[t][0] +
                                  plan.spmms[k].B[t][1] for k in contribs)
                        psum = pp.tile([P, d], F32)
                        mi = 0
                        for k in contribs:
                            sp = plan.spmms[k]
                            bb_lo, go_lo, bb_hi, go_hi = sp.tinfo[t]
                            blo, bhi = sp.B[t]
                            nb = blo + bhi
                            # val-scaled one-hot for ALL nb batches of this
                            # (spmm, tile) in two broadcast DVE ops
                            vh = vp.tile([P, nb * P], BF16)
                            vh3 = vh[:].rearrange("p (b f) -> p b f", f=P)
                            io3 = iota_sb[:, 0:nb * P].rearrange(
                                "p (b f) -> p b f", f=P)
                            nc.vector.tensor_tensor(
                                vh3, io3,
                                rloc_sb[k][:, bb_lo:bb_lo + nb].to_broadcast(
                                    (P, nb, P)),
                                op=ALU.is_equal)
                            nc.vector.tensor_tensor(
                                vh3, vh3,
                                vals_sb[k][:, bb_lo:bb_lo + nb].to_broadcast(
                                    (P, nb, P)),
                                op=ALU.mult)
                            for h, nbh, go0, boff in ((0, blo, go_lo, 0),
                                                      (1, bhi, go_hi, blo)):
                                for bi in range(nbh):
                                    nc.tensor.matmul(
                                        psum[:], vh3[:, boff + bi, :],
                                        gt[(k, h)][:, go0 + bi, :],
                                        start=(mi == 0),
                                        stop=(mi == nmm - 1))
                                    mi += 1
                        if s < cfg.n_step - 1:
                            osb = op.tile([P, d], BF16, tag="osb")
                            nc.vector.tensor_copy(osb[:], psum[:])
                            nc.sync.dma_start(
                                shards[s + 1][ts(t, P), :], osb[:])
                        else:
                            _ln_gelu(nc, op, psum, gamma_sb, beta_sb,
                                     eps_sb, half_sb, out_d, t, cfg)
            if s < cfg.n_step - 1:
                nc.gpsimd.collective_compute(
                    "AllGather", ALU.bypass, replica_groups=RG,
                    ins=[shards[s + 1][0:hs, :]],
                    outs=[tabs[s + 1][0][:, :]])
                nc.gpsimd.collective_compute(
                    "AllGather", ALU.bypass, replica_groups=RG,
                    ins=[shards[s + 1][hs:lp, :]],
                    outs=[tabs[s + 1][1][:, :]])

    # Bacc.compile (via finalize) legalizes multi-waits into event
    # semaphores, auto-inserts gpsimd library loads for dma_gather, and
    # populates extended-ISA instruction bytes.
    nc.finalize()
    return nc


def _ln_gelu(nc, pool, psum, gamma_sb, beta_sb, eps_sb, half_sb, out_d, t,
             cfg: Cfg):
    d = cfg.d
    y = pool.tile([P, d], F32, tag="ln_y")
    nc.vector.tensor_copy(y[:], psum[:])
    negmu = pool.tile([P, 1], F32, tag="ln_mu")
    nc.vector.tensor_reduce(negmu[:], y[:], axis=AX.X, op=ALU.add)
    nc.scalar.mul(negmu[:], negmu[:], -1.0 / d)
    nc.scalar.add(y[:], y[:], negmu[:])  # y = centered
    sq = pool.tile([P, d], F32, tag="ln_sq")
    nc.scalar.activation(sq[:], y[:], AF.Square)
    var = pool.tile([P, 1], F32, tag="ln_var")
    nc.vector.tensor_reduce(var[:], sq[:], axis=AX.X, op=ALU.add)
    istd = pool.tile([P, 1], F32, tag="ln_istd")
    nc.scalar.activation(istd[:], var[:], AF.Sqrt, bias=eps_sb[:],
                         scale=1.0 / d)
    nc.vector.reciprocal(out=istd[:], in_=istd[:])
    nc.scalar.mul(y[:], y[:], istd[:])  # ACT: per-partition scale
    nc.vector.tensor_mul(y[:], y[:], gamma_sb[:])
    nc.vector.tensor_add(y[:], y[:], beta_sb[:])  # y = ln output
    er = pool.tile([P, d], F32, tag="ln_er")
    if cfg.gelu == "erf":
        nc.scalar.activation(er[:], y[:], AF.Erf,
                             scale=float(1.0 / np.sqrt(2.0)))
    else:  # tanh approx (CoreSim has no Erf/Gelu)
        nc.scalar.activation(sq[:], y[:], AF.Square)
        nc.vector.tensor_scalar(sq[:], sq[:], 0.044715, 1.0,
                                op0=ALU.mult, op1=ALU.add)
        nc.vector.tensor_mul(sq[:], sq[:], y[:])
        nc.scalar.activation(er[:], sq[:], AF.Tanh,
                             scale=float(np.sqrt(2.0 / np.pi)))
    # (er + 1) * 0.5 on ACT: 0.5*er + 0.5
    nc.scalar.activation(er[:], er[:], AF.Identity, bias=half_sb[:],
                         scale=0.5)
    nc.vector.tensor_mul(er[:], er[:], y[:])
    nc.sync.dma_start(out_d[ts(t, P), :], er[:])


# ---------------------------------------------------------------------------
# entry point
# ---------------------------------------------------------------------------


def run_on_hw(plan, in_maps, trace=False, **kw):
    nc = build_program(plan)
    cfg = plan.cfg
    res = run_bass_kernel_spmd(
        nc, in_maps, core_ids=list(range(cfg.n_cores)), trace=trace, **kw)
    outs = [res.results[m]["out"] for m in range(cfg.n_cores)]
    full = np.concatenate([o[: cfg.rpc] for o in outs], axis=0)[: cfg.n_nodes]
    return np.ascontiguousarray(full.astype(np.float32)), res


def kernel(**inputs):
    cfg = Cfg()
    plan, in_maps = make_plan_and_inputs(inputs, cfg)
    out, _ = run_on_hw(plan, in_maps)
    return out


# revision 46
# speedup vs baseline: 1.0989x; 1.0989x over previous
"""GNN message-passing kernel for Trainium2, sharded over 8 NeuronCores.

Strategy (matches the "shard nodes by destination row" plan):
  * Nodes (rows of x / segment_sum outputs) are sharded across the 8 cores.
  * h = x @ W.T + b is computed shard-locally on the PE, then AllGathered so
    every core holds the full node table (bf16).
  * Each spmm is executed as, per 128-row destination tile:
      - dma_gather of the source rows x[cols] (bf16, 512B rows) from the
        all-gathered table in HBM into SBUF, edges pre-sorted by dest tile.
      - a per-batch [128 edges x 128 slots] "val-scaled one-hot" matrix built
        on the DVE in one fused tensor_scalar (is_equal -> mult) op.
      - PE matmul psum[slot, :] += onehot.T @ gathered accumulating all edge
        batches of the tile (the segment-sum).
  * Steps are separated by AllGathers of the freshly-computed state shard.
  * The last state stays in fp32 PSUM and goes through LayerNorm + exact-erf
    GELU before being written to the output shard.

All adjacency preprocessing (edge partitioning by destination, sorting,
padding to 128-edge batches, int16 index packing for dma_gather) happens on
the host in numpy inside kernel().
"""

import math
import os
import sys
from contextlib import ExitStack
from dataclasses import dataclass, field

import numpy as np

_TRN_REPO = "/opt/trn_rl_repo"
if _TRN_REPO not in sys.path and not any("trn_rl_repo" in p for p in sys.path):
    sys.path.insert(0, _TRN_REPO)

import ml_dtypes  # noqa: E402

import concourse.bass as bass  # noqa: E402
import concourse.bacc as bacc  # noqa: E402
import concourse.mybir as mybir  # noqa: E402
import concourse.tile as tile  # noqa: E402
from concourse.bass import ts  # noqa: E402
from concourse.bass_utils import run_bass_kernel_spmd  # noqa: E402

F32 = mybir.dt.float32
BF16 = mybir.dt.bfloat16
I16 = mybir.dt.int16
AF = mybir.ActivationFunctionType
ALU = mybir.AluOpType
AX = mybir.AxisListType

LN_EPS = 1e-5
P = 128  # partitions / tile rows
DMA_SCRATCH = 16384  # SWDGE descriptor carveout bytes/partition


@dataclass
class Cfg:
    n_nodes: int = 50000
    d: int = 256
    n_cores: int = 8
    n_step: int = 3
    half: int = 32768  # int16 index range for dma_gather
    # gather group sizes (dest tiles per dma_gather call) per step
    group_sizes: tuple = (6, 3, 2)
    gelu: str = "erf"  # "erf" (exact, HW), "tanh" (sim fallback)
    n_queues: int = 4  # SWDGE descriptor queues for dma_gather

    @property
    def rpc(self):  # rows per core
        return (self.n_nodes + self.n_cores - 1) // self.n_cores

    @property
    def tpc(self):  # 128-row tiles per core
        return (self.rpc + P - 1) // P

    @property
    def lp(self):  # padded local rows
        return self.tpc * P

    @property
    def ntot(self):  # padded total rows (all-gathered table size)
        return self.lp * self.n_cores


# ---------------------------------------------------------------------------
# host-side preprocessing
# ---------------------------------------------------------------------------


@dataclass
class SpmmPlan:
    step: int
    src: int  # 0 = h, 1 = state1, 2 = state2
    # per dest tile: (Blo, Bhi) batch counts (identical across cores)
    B: list = field(default_factory=list)
    TB: int = 0  # total batches = sum(Blo+Bhi)
    idx_cols: int = 0
    # per group: list of ((c0_lo, GBlo), (c0_hi, GBhi)) idx column starts
    calls: list = field(default_factory=list)
    # per tile: (bb0_lo, goff_lo, bb0_hi, goff_hi); goff = batch offset inside
    # the (group, half) gathered buffer
    tinfo: list = field(default_factory=list)


@dataclass
class Plan:
    cfg: Cfg
    spmms: list  # list[SpmmPlan]
    groups: list  # per step: list of list of dest-tile indices
    steps: list  # per step: list of spmm indices


def _pack_positions(g, cfg):
    """global node id -> (half, row) in the split half-tables.

    Each core's rows are split at hsplit = lp/2; half-0 rows of all cores
    form the "lo" table (AllGather #1), half-1 rows the "hi" table
    (AllGather #2). Both tables have n_cores*hsplit rows < 32768, so the
    int16 dma_gather indices cover them."""
    hs = cfg.lp // 2
    m = g // cfg.rpc
    r = g - m * cfg.rpc
    half = (r >= hs).astype(np.int64)
    return half, m * hs + (r - half * hs)


def make_plan_and_inputs(inputs, cfg: Cfg):
    x = np.asarray(inputs["x"], dtype=np.float32)
    adj_rows = np.asarray(inputs["adj_rows"])
    adj_cols = np.asarray(inputs["adj_cols"])
    adj_vals = np.asarray(inputs["adj_vals"], dtype=np.float32)
    idxes_seq = np.asarray(inputs["idxes_seq"]).astype(np.int64)
    idxes_res = np.asarray(inputs["idxes_res"]).astype(np.int64)
    W = np.asarray(inputs["W"], dtype=np.float32)
    b = np.asarray(inputs["b"], dtype=np.float32)
    gamma = np.asarray(inputs["gamma"], dtype=np.float32)
    beta = np.asarray(inputs["beta"], dtype=np.float32)

    nc_, d, tpc = cfg.n_cores, cfg.d, cfg.tpc

    # spmm list: (step, adj_idx, src_state)
    spmm_defs = []
    off = 0
    for i in range(cfg.n_step):
        spmm_defs.append((i, int(idxes_seq[i]), i))
        for j in range(i):
            spmm_defs.append((i, int(idxes_res[off + j]), j))
        off += i
    # order inside a step: seq first then res (matches construction order)
    steps = [[] for _ in range(cfg.n_step)]
    for k, (s, _, _) in enumerate(spmm_defs):
        steps[s].append(k)

    groups = []
    for s in range(cfg.n_step):
        gsz = cfg.group_sizes[s]
        groups.append([list(range(t0, min(t0 + gsz, tpc)))
                      for t0 in range(0, tpc, gsz)])

    # ---- bucket the edges --------------------------------------------------
    # per spmm, per core: sorted arrays + counts
    percore = []  # [k][m] -> dict(i16, rl, v, counts[t,h])
    spmms = []
    for k, (s, a, src) in enumerate(spmm_defs):
        rows = adj_rows[a].astype(np.int64)
        cols = adj_cols[a].astype(np.int64)
        vals = adj_vals[a]
        owner = rows // cfg.rpc
        half_all, ps_all = _pack_positions(cols, cfg)
        cores = []
        counts_all = np.zeros((nc_, tpc, 2), dtype=np.int64)
        for m in range(nc_):
            mask = owner == m
            lr = rows[mask] - m * cfg.rpc
            t = lr // P
            rl = (lr % P).astype(np.float32)
            h = half_all[mask]
            i16 = ps_all[mask].astype(np.int16)
            v = vals[mask]
            key = t * 2 + h
            order = np.argsort(key, kind="stable")
            key = key[order]
            cnt = np.bincount(key, minlength=tpc * 2).reshape(tpc, 2)
            counts_all[m] = cnt
            cores.append(dict(i16=i16[order], rl=rl[order], v=v[order],
                              key=key))
        cmax = counts_all.max(axis=0)  # [tpc, 2]
        B = []
        for t in range(tpc):
            blo = max(1, math.ceil(cmax[t, 0] / P))
            bhi = math.ceil(cmax[t, 1] / P)
            B.append((blo, bhi))
        sp = SpmmPlan(step=s, src=src, B=B)
        sp.TB = sum(bl + bh for bl, bh in B)
        # idx layout: per group: [lo buckets t-major][hi buckets t-major]
        # meta layout: per group: t-major: [lo batches][hi batches]
        calls = []
        c0 = 0
        for g_ts in groups[s]:
            entry = []
            for h in (0, 1):
                GB = sum(B[t][h] for t in g_ts)
                entry.append((c0, GB))
                c0 += GB * 8
            calls.append(entry)
        sp.calls = calls
        sp.idx_cols = c0
        tinfo = [None] * tpc
        bb = 0
        for g_ts in groups[s]:
            golo = 0
            gohi = 0
            # meta order within group: t asc, lo then hi
            for t in g_ts:
                bl, bh = B[t]
                tinfo[t] = (bb, golo, bb + bl, gohi)
                bb += bl + bh
                golo += bl
                gohi += bh
        sp.tinfo = tinfo
        spmms.append(sp)
        percore.append(cores)

    plan = Plan(cfg=cfg, spmms=spmms, groups=groups, steps=steps)
    plan.maxnb = max(bl + bh for sp in spmms for (bl, bh) in sp.B)

    # ---- per-core input arrays --------------------------------------------
    iota = np.broadcast_to(
        np.tile(np.arange(P, dtype=np.float32), plan.maxnb).astype(
            ml_dtypes.bfloat16), (P, plan.maxnb * P)).copy()

    in_maps = []
    for m in range(nc_):
        im = {}
        # xT shard: [d, lp] bf16 (local rows, natural order)
        r0 = m * cfg.rpc
        r1 = min((m + 1) * cfg.rpc, cfg.n_nodes)
        xs = np.zeros((cfg.lp, d), dtype=np.float32)
        xs[: r1 - r0] = x[r0:r1]
        im["xT"] = np.ascontiguousarray(xs.T).astype(ml_dtypes.bfloat16)
        im["WT"] = np.ascontiguousarray(W.T).astype(ml_dtypes.bfloat16)
        im["bias_bc"] = np.broadcast_to(b, (P, d)).copy()
        im["bias_row"] = b.reshape(1, d).astype(ml_dtypes.bfloat16)
        im["gamma_bc"] = np.broadcast_to(gamma, (P, d)).copy()
        im["beta_bc"] = np.broadcast_to(beta, (P, d)).copy()
        im["iota"] = iota

        for k, sp in enumerate(spmms):
            cd = percore[k][m]
            bounds = np.searchsorted(cd["key"], np.arange(tpc * 2 + 1))
            # --- idx array (call order: group -> half -> t) ---
            idx_chunks = []
            for gi, g_ts in enumerate(plan.groups[sp.step]):
                for h in (0, 1):
                    for t in g_ts:
                        Bn = sp.B[t][h]
                        if Bn == 0:
                            continue
                        lo_, hi_ = bounds[t * 2 + h], bounds[t * 2 + h + 1]
                        seg = cd["i16"][lo_:hi_]
                        padv = seg[-1] if len(seg) else np.int16(0)
                        pad = np.full(Bn * P - len(seg), padv, dtype=np.int16)
                        idx_chunks.append(np.concatenate([seg, pad]))
            flat = np.concatenate(idx_chunks) if idx_chunks else np.zeros(
                0, np.int16)
            cols = flat.reshape(-1, 16).T  # [16, cols]
            im[f"idx{k}"] = np.tile(cols, (8, 1)).copy()
            # --- meta arrays (order: group -> t -> lo,hi) ---
            rl_chunks = []
            v_chunks = []
            for gi, g_ts in enumerate(plan.groups[sp.step]):
                for t in g_ts:
                    for h in (0, 1):
                        Bn = sp.B[t][h]
                        if Bn == 0:
                            continue
                        lo_, hi_ = bounds[t * 2 + h], bounds[t * 2 + h + 1]
                        npad = Bn * P - (hi_ - lo_)
                        rl_chunks.append(np.concatenate(
                            [cd["rl"][lo_:hi_],
                             np.zeros(npad, np.float32)]))
                        v_chunks.append(np.concatenate(
                            [cd["v"][lo_:hi_], np.zeros(npad, np.float32)]))
            rl_flat = np.concatenate(rl_chunks)
            v_flat = np.concatenate(v_chunks)
            im[f"rloc{k}"] = np.ascontiguousarray(
                rl_flat.reshape(sp.TB, P).T).astype(ml_dtypes.bfloat16)
            im[f"vals{k}"] = np.ascontiguousarray(
                v_flat.reshape(sp.TB, P).T).astype(ml_dtypes.bfloat16)
        in_maps.append(im)

    return plan, in_maps


# ---------------------------------------------------------------------------
# device program
# ---------------------------------------------------------------------------


def _patch_lane_by_queue(n_queues):
    """Pin Tile's DMASW completion-sem lanes to SWDGE queues.

    The ucode locks each completion semaphore to one SWDGE queue; Tile's
    default round-robin over all 8 lanes ignores queue_num and mixes them.
    Give each queue a dedicated block of lanes instead."""
    from concourse import tile_sem_assignment as tsa
    if getattr(tsa.TileClockTick, "_gnn_patched", 0) == n_queues:
        return
    orig = getattr(tsa.TileClockTick, "_gnn_orig_assign_tick",
                   tsa.TileClockTick._assign_tick)

    def patched(self, inst):
        qn = getattr(inst, "queue_num", None)
        if (qn is not None and inst.engine == mybir.EngineType.Pool
                and isinstance(inst, tsa.DMAInst)):
            if not hasattr(self, "_gnn_q_rr"):
                self._gnn_q_rr = {}
            lpq = max(1, self.swdge_sem_count // n_queues)
            r = self._gnn_q_rr.get(qn, 0)
            self._gnn_q_rr[qn] = (r + 1) % lpq
            self.next_sw_dma_idx = (qn * lpq + r) % self.swdge_sem_count
        return orig(self, inst)

    tsa.TileClockTick._gnn_orig_assign_tick = orig
    tsa.TileClockTick._assign_tick = patched
    tsa.TileClockTick._gnn_patched = n_queues


def build_program(plan: Plan):
    cfg = plan.cfg
    _patch_lane_by_queue(cfg.n_queues)
    d, tpc, lp, ntot = cfg.d, cfg.tpc, cfg.lp, cfg.ntot
    nc = bacc.Bacc("TRN2", target_bir_lowering=False, debug=False,
                   num_devices=cfg.n_cores,
                   dynamic_dma_scratch_size=DMA_SCRATCH,
                   num_swdge_queues=cfg.n_queues)

    hs = lp // 2
    nt2 = hs * cfg.n_cores
    xT = nc.dram_tensor("xT", [d, lp], BF16, kind="ExternalInput")
    WT = nc.dram_tensor("WT", [d, d], BF16, kind="ExternalInput")
    bias_bc = nc.dram_tensor("bias_bc", [P, d], F32, kind="ExternalInput")
    bias_row = nc.dram_tensor("bias_row", [1, d], BF16, kind="ExternalInput")
    gamma_bc = nc.dram_tensor("gamma_bc", [P, d], F32, kind="ExternalInput")
    beta_bc = nc.dram_tensor("beta_bc", [P, d], F32, kind="ExternalInput")
    iota_d = nc.dram_tensor("iota", [P, plan.maxnb * P], BF16,
                            kind="ExternalInput")
    idx_d, rloc_d, vals_d = [], [], []
    for k, sp in enumerate(plan.spmms):
        idx_d.append(nc.dram_tensor(f"idx{k}", [P, sp.idx_cols], I16,
                                    kind="ExternalInput"))
        rloc_d.append(nc.dram_tensor(f"rloc{k}", [P, sp.TB], BF16,
                                     kind="ExternalInput"))
        vals_d.append(nc.dram_tensor(f"vals{k}", [P, sp.TB], BF16,
                                     kind="ExternalInput"))
    out_d = nc.dram_tensor("out", [lp, d], F32, kind="ExternalOutput")

    # every state is computed as a shard then all-gathered into split
    # half-tables via two pipelined AllGathers
    shards = [nc.dram_tensor(f"s{j}_shard", [lp, d], BF16)
              for j in range(cfg.n_step)]
    tabs = [(nc.dram_tensor(f"s{j}_lo", [nt2, d], BF16,
                            addr_space="Shared"),
             nc.dram_tensor(f"s{j}_hi", [nt2, d], BF16,
                            addr_space="Shared"))
            for j in range(cfg.n_step)]
    RG = [list(range(cfg.n_cores))]

    with ExitStack() as ctx:
        tc = ctx.enter_context(tile.TileContext(nc, num_cores=cfg.n_cores))
        const = ctx.enter_context(tc.tile_pool(name="const", bufs=1))

        iota_sb = const.tile([P, plan.maxnb * P], BF16)
        nc.sync.dma_start(iota_sb[:], iota_d[:, :])
        bias_sb = const.tile([P, d], F32)
        nc.sync.dma_start(bias_sb[:], bias_bc[:, :])
        gamma_sb = const.tile([P, d], F32)
        nc.sync.dma_start(gamma_sb[:], gamma_bc[:, :])
        beta_sb = const.tile([P, d], F32)
        nc.sync.dma_start(beta_sb[:], beta_bc[:, :])
        eps_sb = const.tile([P, 1], F32)
        nc.vector.memset(eps_sb[:], LN_EPS)
        half_sb = const.tile([P, 1], F32)
        nc.vector.memset(half_sb[:], 0.5)

        # ---------------- phase: h = x @ W.T + b (shard + split AGs) ------
        # bias folded into the PSUM accumulation via a K=1 ones x b matmul.
        ones_sb = const.tile([1, P], BF16)
        nc.vector.memset(ones_sb[:], 1.0)
        brow_sb = const.tile([1, d], BF16)
        nc.sync.dma_start(brow_sb[:], bias_row[:, :])
        with tc.tile_pool(name="hph", bufs=1) as hp, \
                tc.tile_pool(name="hpsum", bufs=8, space="PSUM") as psh, \
                tc.tile_pool(name="hout", bufs=6) as hop:
            wt0 = hp.tile([P, d], BF16, tag="wt0")
            nc.sync.dma_start(wt0[:], WT[0:P, :])
            wt1 = hp.tile([P, d], BF16, tag="wt1")
            nc.sync.dma_start(wt1[:], WT[P:2 * P, :])
            xt0 = hp.tile([P, lp], BF16, tag="xt0")
            nc.sync.dma_start(xt0[:], xT[0:P, :])
            xt1 = hp.tile([P, lp], BF16, tag="xt1")
            nc.sync.dma_start(xt1[:], xT[P:2 * P, :])
            for t in range(tpc):
                psum = psh.tile([P, d], F32)
                nc.tensor.matmul(psum[:], xt0[:, ts(t, P)], wt0[:],
                                 start=True, stop=False)
                nc.tensor.matmul(psum[:], xt1[:, ts(t, P)], wt1[:],
                                 start=False, stop=False)
                nc.tensor.matmul(psum[:], ones_sb[:], brow_sb[:],
                                 start=False, stop=True)
                hsb = hop.tile([P, d], BF16)
                nc.vector.tensor_copy(hsb[:], psum[:])
                nc.sync.dma_start(shards[0][ts(t, P), :], hsb[:])
        nc.gpsimd.collective_compute(
            "AllGather", ALU.bypass, replica_groups=RG,
            ins=[shards[0][0:hs, :]], outs=[tabs[0][0][:, :]])
        nc.gpsimd.collective_compute(
            "AllGather", ALU.bypass, replica_groups=RG,
            ins=[shards[0][hs:lp, :]], outs=[tabs[0][1][:, :]])

        # ---------------- spmm steps --------------------------------------
        for s in range(cfg.n_step):
            contribs = plan.steps[s]
            maxgb = [[1, 1] for _ in contribs]
            for ci, k in enumerate(contribs):
                for entry in plan.spmms[k].calls:
                    for h in (0, 1):
                        maxgb[ci][h] = max(maxgb[ci][h], entry[h][1])
            with ExitStack() as sctx:
                mp = sctx.enter_context(
                    tc.tile_pool(name=f"meta{s}", bufs=1))
                ip = sctx.enter_context(
                    tc.tile_pool(name=f"idxp{s}", bufs=3))
                gp = sctx.enter_context(
                    tc.tile_pool(name=f"gath{s}", bufs=2))
                vp = sctx.enter_context(
                    tc.tile_pool(name=f"vh{s}", bufs=3))
                pp = sctx.enter_context(
                    tc.tile_pool(name=f"ps{s}", bufs=8, space="PSUM"))
                op = sctx.enter_context(
                    tc.tile_pool(name=f"so{s}", bufs=3))

                rloc_sb, vals_sb = {}, {}
                maxixg = {}
                for k in contribs:
                    sp = plan.spmms[k]
                    rloc_sb[k] = mp.tile([P, sp.TB], BF16, tag=f"rl{k}",
                                         name=f"rl{k}")
                    nc.sync.dma_start(rloc_sb[k][:], rloc_d[k][:, :])
                    vals_sb[k] = mp.tile([P, sp.TB], BF16, tag=f"vl{k}",
                                         name=f"vl{k}")
                    nc.sync.dma_start(vals_sb[k][:], vals_d[k][:, :])
                    maxixg[k] = max((c[0][1] + c[1][1]) * 8
                                    for c in sp.calls)

                nreg = nc.gpsimd.alloc_register(f"nidx{s}")
                qctr = 0
                for gi, g_ts in enumerate(plan.groups[s]):
                    gt = {}
                    for ci, k in enumerate(contribs):
                        sp = plan.spmms[k]
                        (c0_lo, GBlo), (c0_hi, GBhi) = sp.calls[gi]
                        cols_g = (GBlo + GBhi) * 8
                        ixt = ip.tile([P, maxixg[k]], I16, tag=f"ixg{k}",
                                      name=f"ixg{k}")
                        nc.sync.dma_start(ixt[:, 0:cols_g],
                                          idx_d[k][:, c0_lo:c0_lo + cols_g])
                        for h, GB, cg0 in ((0, GBlo, 0), (1, GBhi, GBlo * 8)):
                            if GB == 0:
                                continue
                            g_tile = gp.tile([P, maxgb[ci][h], d], BF16,
                                             tag=f"g{k}_{h}")
                            in_ap = tabs[sp.src][h][:, :]
                            nc.gpsimd.reg_mov(nreg, GB * P)
                            nc.gpsimd.dma_gather(
                                g_tile[:, 0:GB, :], in_ap,
                                ixt[:, cg0:cg0 + GB * 8],
                                num_idxs=GB * P, num_idxs_reg=nreg,
                                elem_size=d,
                                single_packet=(GB * P <= 1024),
                                queue_num=qctr % cfg.n_queues)
                            qctr += 1
                            gt[(k, h)] = g_tile
                    for t in g_ts:
                        nmm = sum(plan.spmms[k].B[t][0] +
                                  plan.spmms[k].B[t][1] for k in contribs)
                        psum = pp.tile([P, d], F32)
                        mi = 0
                        for k in contribs:
                            sp = plan.spmms[k]
                            bb_lo, go_lo, bb_hi, go_hi = sp.tinfo[t]
                            blo, bhi = sp.B[t]
                            nb = blo + bhi
                            # val-scaled one-hot for ALL nb batches of this
                            # (spmm, tile) in two broadcast DVE ops
                            vh = vp.tile([P, nb * P], BF16)
                            vh3 = vh[:].rearrange("p (b f) -> p b f", f=P)
                            io3 = iota_sb[:, 0:nb * P].rearrange(
                                "p (b f) -> p b f", f=P)
                            nc.vector.tensor_tensor(
                                vh3, io3,
                                rloc_sb[k][:, bb_lo:bb_lo + nb].to_broadcast(
                                    (P, nb, P)),
                                op=ALU.is_equal)
                            nc.vector.tensor_tensor(
                                vh3, vh3,
                                vals_sb[k][:, bb_lo:bb_lo + nb].to_broadcast(
                                    (P, nb, P)),
                                op=ALU.mult)
                            for h, nbh, go0, boff in ((0, blo, go_lo, 0),
                                                      (1, bhi, go_hi, blo)):
                                for bi in range(nbh):
                                    nc.tensor.matmul(
                                        psum[:], vh3[:, boff + bi, :],
                                        gt[(k, h)][:, go0 + bi, :],
                                        start=(mi == 0),
                                        stop=(mi == nmm - 1))
                                    mi += 1
                        if s < cfg.n_step - 1:
                            osb = op.tile([P, d], BF16, tag="osb")
                            nc.vector.tensor_copy(osb[:], psum[:])
                            nc.sync.dma_start(
                                shards[s + 1][ts(t, P), :], osb[:])
                        else:
                            _ln_gelu(nc, op, psum, gamma_sb, beta_sb,
                                     eps_sb, half_sb, out_d, t, cfg)
            if s < cfg.n_step - 1:
                nc.gpsimd.collective_compute(
                    "AllGather", ALU.bypass, replica_groups=RG,
                    ins=[shards[s + 1][0:hs, :]],
                    outs=[tabs[s + 1][0][:, :]])
                nc.gpsimd.collective_compute(
                    "AllGather", ALU.bypass, replica_groups=RG,
                    ins=[shards[s + 1][hs:lp, :]],
                    outs=[tabs[s + 1][1][:, :]])

    # Bacc.compile (via finalize) legalizes multi-waits into event
    # semaphores, auto-inserts gpsimd library loads for dma_gather, and
    # populates extended-ISA instruction bytes.
    nc.finalize()
    return nc


def _ln_gelu(nc, pool, psum, gamma_sb, beta_sb, eps_sb, half_sb, out_d, t,
             cfg: Cfg):
    d = cfg.d
    y = pool.tile([P, d], F32, tag="ln_y")
    nc.vector.tensor_copy(y[:], psum[:])
    negmu = pool.tile([P, 1], F32, tag="ln_mu")
    nc.vector.tensor_reduce(negmu[:], y[:], axis=AX.X, op=ALU.add)
    nc.scalar.mul(negmu[:], negmu[:], -1.0 / d)
    nc.scalar.add(y[:], y[:], negmu[:])  # y = centered
    sq = pool.tile([P, d], F32, tag="ln_sq")
    nc.scalar.activation(sq[:], y[:], AF.Square)
    var = pool.tile([P, 1], F32, tag="ln_var")
    nc.vector.tensor_reduce(var[:], sq[:], axis=AX.X, op=ALU.add)
    istd = pool.tile([P, 1], F32, tag="ln_istd")
    nc.scalar.activation(istd[:], var[:], AF.Sqrt, bias=eps_sb[:],
                         scale=1.0 / d)
    nc.vector.reciprocal(out=istd[:], in_=istd[:])
    nc.scalar.mul(y[:], y[:], istd[:])  # ACT: per-partition scale
    nc.vector.tensor_mul(y[:], y[:], gamma_sb[:])
    nc.vector.tensor_add(y[:], y[:], beta_sb[:])  # y = ln output
    er = pool.tile([P, d], F32, tag="ln_er")
    if cfg.gelu == "erf":
        nc.scalar.activation(er[:], y[:], AF.Erf,
                             scale=float(1.0 / np.sqrt(2.0)))
    else:  # tanh approx (CoreSim has no Erf/Gelu)
        nc.scalar.activation(sq[:], y[:], AF.Square)
        nc.vector.tensor_scalar(sq[:], sq[:], 0.044715, 1.0,
                                op0=ALU.mult, op1=ALU.add)
        nc.vector.tensor_mul(sq[:], sq[:], y[:])
        nc.scalar.activation(er[:], sq[:], AF.Tanh,
                             scale=float(np.sqrt(2.0 / np.pi)))
    # (er + 1) * 0.5 on ACT: 0.5*er + 0.5
    nc.scalar.activation(er[:], er[:], AF.Identity, bias=half_sb[:],
                         scale=0.5)
    nc.vector.tensor_mul(er[:], er[:], y[:])
    nc.sync.dma_start(out_d[ts(t, P), :], er[:])


# ---------------------------------------------------------------------------
# entry point
# ---------------------------------------------------------------------------


def run_on_hw(plan, in_maps, trace=False, **kw):
    nc = build_program(plan)
    cfg = plan.cfg
    res = run_bass_kernel_spmd(
        nc, in_maps, core_ids=list(range(cfg.n_cores)), trace=trace, **kw)
    outs = [res.results[m]["out"] for m in range(cfg.n_cores)]
    full = np.concatenate([o[: cfg.rpc] for o in outs], axis=0)[: cfg.n_nodes]
    return np.ascontiguousarray(full.astype(np.float32)), res


def kernel(**inputs):
    cfg = Cfg()
    plan, in_maps = make_plan_and_inputs(inputs, cfg)
    out, _ = run_on_hw(plan, in_maps)
    return out


# revision 49
# speedup vs baseline: 1.2110x; 1.1020x over previous
"""GNN message-passing kernel for Trainium2, sharded over 8 NeuronCores.

Strategy (matches the "shard nodes by destination row" plan):
  * Nodes (rows of x / segment_sum outputs) are sharded across the 8 cores.
  * h = x @ W.T + b is computed shard-locally on the PE, then AllGathered so
    every core holds the full node table (bf16).
  * Each spmm is executed as, per 128-row destination tile:
      - dma_gather of the source rows x[cols] (bf16, 512B rows) from the
        all-gathered table in HBM into SBUF, edges pre-sorted by dest tile.
      - a per-batch [128 edges x 128 slots] "val-scaled one-hot" matrix built
        on the DVE in one fused tensor_scalar (is_equal -> mult) op.
      - PE matmul psum[slot, :] += onehot.T @ gathered accumulating all edge
        batches of the tile (the segment-sum).
  * Steps are separated by AllGathers of the freshly-computed state shard.
  * The last state stays in fp32 PSUM and goes through LayerNorm + exact-erf
    GELU before being written to the output shard.

All adjacency preprocessing (edge partitioning by destination, sorting,
padding to 128-edge batches, int16 index packing for dma_gather) happens on
the host in numpy inside kernel().
"""

import math
import os
import sys
from contextlib import ExitStack
from dataclasses import dataclass, field

import numpy as np

_TRN_REPO = "/opt/trn_rl_repo"
if _TRN_REPO not in sys.path and not any("trn_rl_repo" in p for p in sys.path):
    sys.path.insert(0, _TRN_REPO)

import ml_dtypes  # noqa: E402

import concourse.bass as bass  # noqa: E402
import concourse.bacc as bacc  # noqa: E402
import concourse.mybir as mybir  # noqa: E402
import concourse.tile as tile  # noqa: E402
from concourse.bass import ts  # noqa: E402
from concourse.bass_utils import run_bass_kernel_spmd  # noqa: E402

F32 = mybir.dt.float32
BF16 = mybir.dt.bfloat16
I16 = mybir.dt.int16
AF = mybir.ActivationFunctionType
ALU = mybir.AluOpType
AX = mybir.AxisListType

LN_EPS = 1e-5
P = 128  # partitions / tile rows
DMA_SCRATCH = 16384  # SWDGE descriptor carveout bytes/partition


@dataclass
class Cfg:
    n_nodes: int = 50000
    d: int = 256
    n_cores: int = 8
    n_step: int = 3
    half: int = 32768  # int16 index range for dma_gather
    # gather group sizes (dest tiles per dma_gather call) per step
    group_sizes: tuple = (3, 2, 2)
    gelu: str = "erf"  # "erf" (exact, HW), "tanh" (sim fallback)
    n_queues: int = 4  # SWDGE descriptor queues for dma_gather

    @property
    def rpc(self):  # rows per core
        return (self.n_nodes + self.n_cores - 1) // self.n_cores

    @property
    def tpc(self):  # 128-row tiles per core
        return (self.rpc + P - 1) // P

    @property
    def lp(self):  # padded local rows
        return self.tpc * P

    @property
    def ntot(self):  # padded total rows (all-gathered table size)
        return self.lp * self.n_cores


# ---------------------------------------------------------------------------
# host-side preprocessing
# ---------------------------------------------------------------------------


@dataclass
class SpmmPlan:
    step: int
    src: int  # 0 = h, 1 = state1, 2 = state2
    # per dest tile: (Blo, Bhi) batch counts (identical across cores)
    B: list = field(default_factory=list)
    TB: int = 0  # total batches = sum(Blo+Bhi)
    idx_cols: int = 0
    # per group: list of ((c0_lo, GBlo), (c0_hi, GBhi)) idx column starts
    calls: list = field(default_factory=list)
    # per tile: (bb0_lo, goff_lo, bb0_hi, goff_hi); goff = batch offset inside
    # the (group, half) gathered buffer
    tinfo: list = field(default_factory=list)


@dataclass
class Plan:
    cfg: Cfg
    spmms: list  # list[SpmmPlan]
    groups: list  # per step: list of list of dest-tile indices
    steps: list  # per step: list of spmm indices


def _pack_positions(g, cfg):
    """global node id -> (half, row) in the split half-tables.

    Each core's rows are split at hsplit = lp/2; half-0 rows of all cores
    form the "lo" table (AllGather #1), half-1 rows the "hi" table
    (AllGather #2). Both tables have n_cores*hsplit rows < 32768, so the
    int16 dma_gather indices cover them."""
    hs = cfg.lp // 2
    m = g // cfg.rpc
    r = g - m * cfg.rpc
    half = (r >= hs).astype(np.int64)
    return half, m * hs + (r - half * hs)


def make_plan_and_inputs(inputs, cfg: Cfg):
    x = np.asarray(inputs["x"], dtype=np.float32)
    adj_rows = np.asarray(inputs["adj_rows"])
    adj_cols = np.asarray(inputs["adj_cols"])
    adj_vals = np.asarray(inputs["adj_vals"], dtype=np.float32)
    idxes_seq = np.asarray(inputs["idxes_seq"]).astype(np.int64)
    idxes_res = np.asarray(inputs["idxes_res"]).astype(np.int64)
    W = np.asarray(inputs["W"], dtype=np.float32)
    b = np.asarray(inputs["b"], dtype=np.float32)
    gamma = np.asarray(inputs["gamma"], dtype=np.float32)
    beta = np.asarray(inputs["beta"], dtype=np.float32)

    nc_, d, tpc = cfg.n_cores, cfg.d, cfg.tpc

    # spmm list: (step, adj_idx, src_state)
    spmm_defs = []
    off = 0
    for i in range(cfg.n_step):
        spmm_defs.append((i, int(idxes_seq[i]), i))
        for j in range(i):
            spmm_defs.append((i, int(idxes_res[off + j]), j))
        off += i
    # order inside a step: seq first then res (matches construction order)
    steps = [[] for _ in range(cfg.n_step)]
    for k, (s, _, _) in enumerate(spmm_defs):
        steps[s].append(k)

    groups = []
    for s in range(cfg.n_step):
        gsz = cfg.group_sizes[s]
        groups.append([list(range(t0, min(t0 + gsz, tpc)))
                      for t0 in range(0, tpc, gsz)])

    # ---- bucket the edges --------------------------------------------------
    # per spmm, per core: sorted arrays + counts
    percore = []  # [k][m] -> dict(i16, rl, v, counts[t,h])
    spmms = []
    for k, (s, a, src) in enumerate(spmm_defs):
        rows = adj_rows[a].astype(np.int64)
        cols = adj_cols[a].astype(np.int64)
        vals = adj_vals[a]
        owner = rows // cfg.rpc
        half_all, ps_all = _pack_positions(cols, cfg)
        cores = []
        counts_all = np.zeros((nc_, tpc, 2), dtype=np.int64)
        for m in range(nc_):
            mask = owner == m
            lr = rows[mask] - m * cfg.rpc
            t = lr // P
            rl = (lr % P).astype(np.float32)
            h = half_all[mask]
            i16 = ps_all[mask].astype(np.int16)
            v = vals[mask]
            key = t * 2 + h
            order = np.argsort(key, kind="stable")
            key = key[order]
            cnt = np.bincount(key, minlength=tpc * 2).reshape(tpc, 2)
            counts_all[m] = cnt
            cores.append(dict(i16=i16[order], rl=rl[order], v=v[order],
                              key=key))
        cmax = counts_all.max(axis=0)  # [tpc, 2]
        B = []
        for t in range(tpc):
            blo = max(1, math.ceil(cmax[t, 0] / P))
            bhi = math.ceil(cmax[t, 1] / P)
            B.append((blo, bhi))
        sp = SpmmPlan(step=s, src=src, B=B)
        sp.TB = sum(bl + bh for bl, bh in B)
        # idx layout: per group: [lo buckets t-major][hi buckets t-major]
        # meta layout: per group: t-major: [lo batches][hi batches]
        calls = []
        c0 = 0
        for g_ts in groups[s]:
            entry = []
            for h in (0, 1):
                GB = sum(B[t][h] for t in g_ts)
                entry.append((c0, GB))
                c0 += GB * 8
            calls.append(entry)
        sp.calls = calls
        sp.idx_cols = c0
        tinfo = [None] * tpc
        bb = 0
        for g_ts in groups[s]:
            golo = 0
            gohi = 0
            # meta order within group: t asc, lo then hi
            for t in g_ts:
                bl, bh = B[t]
                tinfo[t] = (bb, golo, bb + bl, gohi)
                bb += bl + bh
                golo += bl
                gohi += bh
        sp.tinfo = tinfo
        spmms.append(sp)
        percore.append(cores)

    plan = Plan(cfg=cfg, spmms=spmms, groups=groups, steps=steps)
    plan.maxnb = max(bl + bh for sp in spmms for (bl, bh) in sp.B)

    # ---- per-core input arrays --------------------------------------------
    iota = np.broadcast_to(
        np.tile(np.arange(P, dtype=np.float32), plan.maxnb).astype(
            ml_dtypes.bfloat16), (P, plan.maxnb * P)).copy()

    in_maps = []
    for m in range(nc_):
        im = {}
        # xT shard: [d, lp] bf16 (local rows, natural order)
        r0 = m * cfg.rpc
        r1 = min((m + 1) * cfg.rpc, cfg.n_nodes)
        xs = np.zeros((cfg.lp, d), dtype=np.float32)
        xs[: r1 - r0] = x[r0:r1]
        im["xT"] = np.ascontiguousarray(xs.T).astype(ml_dtypes.bfloat16)
        im["WT"] = np.ascontiguousarray(W.T).astype(ml_dtypes.bfloat16)
        im["bias_bc"] = np.broadcast_to(b, (P, d)).copy()
        im["bias_row"] = b.reshape(1, d).astype(ml_dtypes.bfloat16)
        im["gamma_bc"] = np.broadcast_to(gamma, (P, d)).copy()
        im["beta_bc"] = np.broadcast_to(beta, (P, d)).copy()
        im["iota"] = iota

        for k, sp in enumerate(spmms):
            cd = percore[k][m]
            bounds = np.searchsorted(cd["key"], np.arange(tpc * 2 + 1))
            # --- idx array (call order: group -> half -> t) ---
            idx_chunks = []
            for gi, g_ts in enumerate(plan.groups[sp.step]):
                for h in (0, 1):
                    for t in g_ts:
                        Bn = sp.B[t][h]
                        if Bn == 0:
                            continue
                        lo_, hi_ = bounds[t * 2 + h], bounds[t * 2 + h + 1]
                        seg = cd["i16"][lo_:hi_]
                        padv = seg[-1] if len(seg) else np.int16(0)
                        pad = np.full(Bn * P - len(seg), padv, dtype=np.int16)
                        idx_chunks.append(np.concatenate([seg, pad]))
            flat = np.concatenate(idx_chunks) if idx_chunks else np.zeros(
                0, np.int16)
            cols = flat.reshape(-1, 16).T  # [16, cols]
            im[f"idx{k}"] = np.tile(cols, (8, 1)).copy()
            # --- meta arrays (order: group -> t -> lo,hi) ---
            rl_chunks = []
            v_chunks = []
            for gi, g_ts in enumerate(plan.groups[sp.step]):
                for t in g_ts:
                    for h in (0, 1):
                        Bn = sp.B[t][h]
                        if Bn == 0:
                            continue
                        lo_, hi_ = bounds[t * 2 + h], bounds[t * 2 + h + 1]
                        npad = Bn * P - (hi_ - lo_)
                        rl_chunks.append(np.concatenate(
                            [cd["rl"][lo_:hi_],
                             np.zeros(npad, np.float32)]))
                        v_chunks.append(np.concatenate(
                            [cd["v"][lo_:hi_], np.zeros(npad, np.float32)]))
            rl_flat = np.concatenate(rl_chunks)
            v_flat = np.concatenate(v_chunks)
            im[f"rloc{k}"] = np.ascontiguousarray(
                rl_flat.reshape(sp.TB, P).T).astype(ml_dtypes.bfloat16)
            im[f"vals{k}"] = np.ascontiguousarray(
                v_flat.reshape(sp.TB, P).T).astype(ml_dtypes.bfloat16)
        in_maps.append(im)

    return plan, in_maps


# ---------------------------------------------------------------------------
# device program
# ---------------------------------------------------------------------------


def _patch_lane_by_queue(n_queues):
    """Pin Tile's DMASW completion-sem lanes to SWDGE queues.

    The ucode locks each completion semaphore to one SWDGE queue; Tile's
    default round-robin over all 8 lanes ignores queue_num and mixes them.
    Give each queue a dedicated block of lanes instead."""
    from concourse import tile_sem_assignment as tsa
    if getattr(tsa.TileClockTick, "_gnn_patched", 0) == n_queues:
        return
    orig = getattr(tsa.TileClockTick, "_gnn_orig_assign_tick",
                   tsa.TileClockTick._assign_tick)

    def patched(self, inst):
        qn = getattr(inst, "queue_num", None)
        if (qn is not None and inst.engine == mybir.EngineType.Pool
                and isinstance(inst, tsa.DMAInst)):
            if not hasattr(self, "_gnn_q_rr"):
                self._gnn_q_rr = {}
            lpq = max(1, self.swdge_sem_count // n_queues)
            r = self._gnn_q_rr.get(qn, 0)
            self._gnn_q_rr[qn] = (r + 1) % lpq
            self.next_sw_dma_idx = (qn * lpq + r) % self.swdge_sem_count
        return orig(self, inst)

    tsa.TileClockTick._gnn_orig_assign_tick = orig
    tsa.TileClockTick._assign_tick = patched
    tsa.TileClockTick._gnn_patched = n_queues


def build_program(plan: Plan):
    cfg = plan.cfg
    _patch_lane_by_queue(cfg.n_queues)
    d, tpc, lp, ntot = cfg.d, cfg.tpc, cfg.lp, cfg.ntot
    nc = bacc.Bacc("TRN2", target_bir_lowering=False, debug=False,
                   num_devices=cfg.n_cores,
                   dynamic_dma_scratch_size=DMA_SCRATCH,
                   num_swdge_queues=cfg.n_queues)

    hs = lp // 2
    nt2 = hs * cfg.n_cores
    xT = nc.dram_tensor("xT", [d, lp], BF16, kind="ExternalInput")
    WT = nc.dram_tensor("WT", [d, d], BF16, kind="ExternalInput")
    bias_bc = nc.dram_tensor("bias_bc", [P, d], F32, kind="ExternalInput")
    bias_row = nc.dram_tensor("bias_row", [1, d], BF16, kind="ExternalInput")
    gamma_bc = nc.dram_tensor("gamma_bc", [P, d], F32, kind="ExternalInput")
    beta_bc = nc.dram_tensor("beta_bc", [P, d], F32, kind="ExternalInput")
    iota_d = nc.dram_tensor("iota", [P, plan.maxnb * P], BF16,
                            kind="ExternalInput")
    idx_d, rloc_d, vals_d = [], [], []
    for k, sp in enumerate(plan.spmms):
        idx_d.append(nc.dram_tensor(f"idx{k}", [P, sp.idx_cols], I16,
                                    kind="ExternalInput"))
        rloc_d.append(nc.dram_tensor(f"rloc{k}", [P, sp.TB], BF16,
                                     kind="ExternalInput"))
        vals_d.append(nc.dram_tensor(f"vals{k}", [P, sp.TB], BF16,
                                     kind="ExternalInput"))
    out_d = nc.dram_tensor("out", [lp, d], F32, kind="ExternalOutput")

    # every state is computed as a shard then all-gathered into split
    # half-tables via two pipelined AllGathers
    shards = [nc.dram_tensor(f"s{j}_shard", [lp, d], BF16)
              for j in range(cfg.n_step)]
    tabs = [(nc.dram_tensor(f"s{j}_lo", [nt2, d], BF16,
                            addr_space="Shared"),
             nc.dram_tensor(f"s{j}_hi", [nt2, d], BF16,
                            addr_space="Shared"))
            for j in range(cfg.n_step)]
    RG = [list(range(cfg.n_cores))]

    with ExitStack() as ctx:
        tc = ctx.enter_context(tile.TileContext(nc, num_cores=cfg.n_cores))
        const = ctx.enter_context(tc.tile_pool(name="const", bufs=1))

        iota_sb = const.tile([P, plan.maxnb * P], BF16)
        nc.sync.dma_start(iota_sb[:], iota_d[:, :])
        bias_sb = const.tile([P, d], F32)
        nc.sync.dma_start(bias_sb[:], bias_bc[:, :])
        gamma_sb = const.tile([P, d], F32)
        nc.sync.dma_start(gamma_sb[:], gamma_bc[:, :])
        beta_sb = const.tile([P, d], F32)
        nc.sync.dma_start(beta_sb[:], beta_bc[:, :])
        eps_sb = const.tile([P, 1], F32)
        nc.vector.memset(eps_sb[:], LN_EPS)
        half_sb = const.tile([P, 1], F32)
        nc.vector.memset(half_sb[:], 0.5)

        # ---------------- phase: h = x @ W.T + b (shard + split AGs) ------
        # bias folded into the PSUM accumulation via a K=1 ones x b matmul.
        ones_sb = const.tile([1, P], BF16)
        nc.vector.memset(ones_sb[:], 1.0)
        brow_sb = const.tile([1, d], BF16)
        nc.sync.dma_start(brow_sb[:], bias_row[:, :])
        with tc.tile_pool(name="hph", bufs=1) as hp, \
                tc.tile_pool(name="hpsum", bufs=8, space="PSUM") as psh, \
                tc.tile_pool(name="hout", bufs=6) as hop:
            wt0 = hp.tile([P, d], BF16, tag="wt0")
            nc.sync.dma_start(wt0[:], WT[0:P, :])
            wt1 = hp.tile([P, d], BF16, tag="wt1")
            nc.sync.dma_start(wt1[:], WT[P:2 * P, :])
            xt0 = hp.tile([P, lp], BF16, tag="xt0")
            nc.sync.dma_start(xt0[:], xT[0:P, :])
            xt1 = hp.tile([P, lp], BF16, tag="xt1")
            nc.sync.dma_start(xt1[:], xT[P:2 * P, :])
            for t in range(tpc):
                psum = psh.tile([P, d], F32)
                nc.tensor.matmul(psum[:], xt0[:, ts(t, P)], wt0[:],
                                 start=True, stop=False)
                nc.tensor.matmul(psum[:], xt1[:, ts(t, P)], wt1[:],
                                 start=False, stop=False)
                nc.tensor.matmul(psum[:], ones_sb[:], brow_sb[:],
                                 start=False, stop=True)
                hsb = hop.tile([P, d], BF16)
                nc.vector.tensor_copy(hsb[:], psum[:])
                nc.sync.dma_start(shards[0][ts(t, P), :], hsb[:])
        nc.gpsimd.collective_compute(
            "AllGather", ALU.bypass, replica_groups=RG,
            ins=[shards[0][0:hs, :]], outs=[tabs[0][0][:, :]])
        nc.gpsimd.collective_compute(
            "AllGather", ALU.bypass, replica_groups=RG,
            ins=[shards[0][hs:lp, :]], outs=[tabs[0][1][:, :]])

        # ---------------- spmm steps --------------------------------------
        for s in range(cfg.n_step):
            contribs = plan.steps[s]
            maxgb = [[1, 1] for _ in contribs]
            for ci, k in enumerate(contribs):
                for entry in plan.spmms[k].calls:
                    for h in (0, 1):
                        maxgb[ci][h] = max(maxgb[ci][h], entry[h][1])
            with ExitStack() as sctx:
                mp = sctx.enter_context(
                    tc.tile_pool(name=f"meta{s}", bufs=1))
                ip = sctx.enter_context(
                    tc.tile_pool(name=f"idxp{s}", bufs=4))
                gp = sctx.enter_context(
                    tc.tile_pool(name=f"gath{s}",
                                 bufs=(4, 3, 2)[min(s, 2)]))
                vp = sctx.enter_context(
                    tc.tile_pool(name=f"vh{s}", bufs=3))
                pp = sctx.enter_context(
                    tc.tile_pool(name=f"ps{s}", bufs=8, space="PSUM"))
                op = sctx.enter_context(
                    tc.tile_pool(name=f"so{s}", bufs=3))

                rloc_sb, vals_sb = {}, {}
                maxixg = {}
                for k in contribs:
                    sp = plan.spmms[k]
                    rloc_sb[k] = mp.tile([P, sp.TB], BF16, tag=f"rl{k}",
                                         name=f"rl{k}")
                    nc.sync.dma_start(rloc_sb[k][:], rloc_d[k][:, :])
                    vals_sb[k] = mp.tile([P, sp.TB], BF16, tag=f"vl{k}",
                                         name=f"vl{k}")
                    nc.sync.dma_start(vals_sb[k][:], vals_d[k][:, :])
                    maxixg[k] = max((c[0][1] + c[1][1]) * 8
                                    for c in sp.calls)

                nreg = nc.gpsimd.alloc_register(f"nidx{s}")
                qctr = 0
                for gi, g_ts in enumerate(plan.groups[s]):
                    gt = {}
                    for ci, k in enumerate(contribs):
                        sp = plan.spmms[k]
                        (c0_lo, GBlo), (c0_hi, GBhi) = sp.calls[gi]
                        cols_g = (GBlo + GBhi) * 8
                        ixt = ip.tile([P, maxixg[k]], I16, tag=f"ixg{k}",
                                      name=f"ixg{k}")
                        nc.sync.dma_start(ixt[:, 0:cols_g],
                                          idx_d[k][:, c0_lo:c0_lo + cols_g])
                        for h, GB, cg0 in ((0, GBlo, 0), (1, GBhi, GBlo * 8)):
                            if GB == 0:
                                continue
                            g_tile = gp.tile([P, maxgb[ci][h], d], BF16,
                                             tag=f"g{k}_{h}")
                            in_ap = tabs[sp.src][h][:, :]
                            nc.gpsimd.reg_mov(nreg, GB * P)
                            nc.gpsimd.dma_gather(
                                g_tile[:, 0:GB, :], in_ap,
                                ixt[:, cg0:cg0 + GB * 8],
                                num_idxs=GB * P, num_idxs_reg=nreg,
                                elem_size=d,
                                single_packet=(GB * P <= 1024),
                                queue_num=(ci * 2 + h) % cfg.n_queues
                                if len(contribs) > 1
                                else qctr % cfg.n_queues)
                            qctr += 1
                            gt[(k, h)] = g_tile
                    for t in g_ts:
                        nmm = sum(plan.spmms[k].B[t][0] +
                                  plan.spmms[k].B[t][1] for k in contribs)
                        psum = pp.tile([P, d], F32)
                        mi = 0
                        for k in contribs:
                            sp = plan.spmms[k]
                            bb_lo, go_lo, bb_hi, go_hi = sp.tinfo[t]
                            blo, bhi = sp.B[t]
                            nb = blo + bhi
                            # val-scaled one-hot for ALL nb batches of this
                            # (spmm, tile) in two broadcast DVE ops
                            vh = vp.tile([P, nb * P], BF16)
                            vh3 = vh[:].rearrange("p (b f) -> p b f", f=P)
                            io3 = iota_sb[:, 0:nb * P].rearrange(
                                "p (b f) -> p b f", f=P)
                            nc.vector.tensor_tensor(
                                vh3, io3,
                                rloc_sb[k][:, bb_lo:bb_lo + nb].to_broadcast(
                                    (P, nb, P)),
                                op=ALU.is_equal)
                            nc.vector.tensor_tensor(
                                vh3, vh3,
                                vals_sb[k][:, bb_lo:bb_lo + nb].to_broadcast(
                                    (P, nb, P)),
                                op=ALU.mult)
                            for h, nbh, go0, boff in ((0, blo, go_lo, 0),
                                                      (1, bhi, go_hi, blo)):
                                for bi in range(nbh):
                                    nc.tensor.matmul(
                                        psum[:], vh3[:, boff + bi, :],
                                        gt[(k, h)][:, go0 + bi, :],
                                        start=(mi == 0),
                                        stop=(mi == nmm - 1))
                                    mi += 1
                        if s < cfg.n_step - 1:
                            osb = op.tile([P, d], BF16, tag="osb")
                            nc.vector.tensor_copy(osb[:], psum[:])
                            nc.sync.dma_start(
                                shards[s + 1][ts(t, P), :], osb[:])
                        else:
                            _ln_gelu(nc, op, psum, gamma_sb, beta_sb,
                                     eps_sb, half_sb, out_d, t, cfg)
            if s < cfg.n_step - 1:
                nc.gpsimd.collective_compute(
                    "AllGather", ALU.bypass, replica_groups=RG,
                    ins=[shards[s + 1][0:hs, :]],
                    outs=[tabs[s + 1][0][:, :]])
                nc.gpsimd.collective_compute(
                    "AllGather", ALU.bypass, replica_groups=RG,
                    ins=[shards[s + 1][hs:lp, :]],
                    outs=[tabs[s + 1][1][:, :]])

    # Bacc.compile (via finalize) legalizes multi-waits into event
    # semaphores, auto-inserts gpsimd library loads for dma_gather, and
    # populates extended-ISA instruction bytes.
    nc.finalize()
    return nc


def _ln_gelu(nc, pool, psum, gamma_sb, beta_sb, eps_sb, half_sb, out_d, t,
             cfg: Cfg):
    d = cfg.d
    y = pool.tile([P, d], F32, tag="ln_y")
    nc.vector.tensor_copy(y[:], psum[:])
    negmu = pool.tile([P, 1], F32, tag="ln_mu")
    nc.vector.tensor_reduce(negmu[:], y[:], axis=AX.X, op=ALU.add)
    nc.scalar.mul(negmu[:], negmu[:], -1.0 / d)
    nc.scalar.add(y[:], y[:], negmu[:])  # y = centered
    sq = pool.tile([P, d], F32, tag="ln_sq")
    nc.scalar.activation(sq[:], y[:], AF.Square)
    var = pool.tile([P, 1], F32, tag="ln_var")
    nc.vector.tensor_reduce(var[:], sq[:], axis=AX.X, op=ALU.add)
    istd = pool.tile([P, 1], F32, tag="ln_istd")
    nc.scalar.activation(istd[:], var[:], AF.Sqrt, bias=eps_sb[:],
                         scale=1.0 / d)
    nc.vector.reciprocal(out=istd[:], in_=istd[:])
    nc.scalar.mul(y[:], y[:], istd[:])  # ACT: per-partition scale
    nc.vector.tensor_mul(y[:], y[:], gamma_sb[:])
    nc.vector.tensor_add(y[:], y[:], beta_sb[:])  # y = ln output
    er = pool.tile([P, d], F32, tag="ln_er")
    if cfg.gelu == "erf":
        nc.scalar.activation(er[:], y[:], AF.Erf,
                             scale=float(1.0 / np.sqrt(2.0)))
    else:  # tanh approx (CoreSim has no Erf/Gelu)
        nc.scalar.activation(sq[:], y[:], AF.Square)
        nc.vector.tensor_scalar(sq[:], sq[:], 0.044715, 1.0,
                                op0=ALU.mult, op1=ALU.add)
        nc.vector.tensor_mul(sq[:], sq[:], y[:])
        nc.scalar.activation(er[:], sq[:], AF.Tanh,
                             scale=float(np.sqrt(2.0 / np.pi)))
    # (er + 1) * 0.5 on ACT: 0.5*er + 0.5
    nc.scalar.activation(er[:], er[:], AF.Identity, bias=half_sb[:],
                         scale=0.5)
    nc.vector.tensor_mul(er[:], er[:], y[:])
    nc.sync.dma_start(out_d[ts(t, P), :], er[:])


# ---------------------------------------------------------------------------
# entry point
# ---------------------------------------------------------------------------


def run_on_hw(plan, in_maps, trace=False, **kw):
    nc = build_program(plan)
    cfg = plan.cfg
    res = run_bass_kernel_spmd(
        nc, in_maps, core_ids=list(range(cfg.n_cores)), trace=trace, **kw)
    outs = [res.results[m]["out"] for m in range(cfg.n_cores)]
    full = np.concatenate([o[: cfg.rpc] for o in outs], axis=0)[: cfg.n_nodes]
    return np.ascontiguousarray(full.astype(np.float32)), res


def kernel(**inputs):
    cfg = Cfg()
    plan, in_maps = make_plan_and_inputs(inputs, cfg)
    out, _ = run_on_hw(plan, in_maps)
    return out


# revision 50
# speedup vs baseline: 1.2396x; 1.0236x over previous
"""GNN message-passing kernel for Trainium2, sharded over 8 NeuronCores.

Strategy (matches the "shard nodes by destination row" plan):
  * Nodes (rows of x / segment_sum outputs) are sharded across the 8 cores.
  * h = x @ W.T + b is computed shard-locally on the PE, then AllGathered so
    every core holds the full node table (bf16).
  * Each spmm is executed as, per 128-row destination tile:
      - dma_gather of the source rows x[cols] (bf16, 512B rows) from the
        all-gathered table in HBM into SBUF, edges pre-sorted by dest tile.
      - a per-batch [128 edges x 128 slots] "val-scaled one-hot" matrix built
        on the DVE in one fused tensor_scalar (is_equal -> mult) op.
      - PE matmul psum[slot, :] += onehot.T @ gathered accumulating all edge
        batches of the tile (the segment-sum).
  * Steps are separated by AllGathers of the freshly-computed state shard.
  * The last state stays in fp32 PSUM and goes through LayerNorm + exact-erf
    GELU before being written to the output shard.

All adjacency preprocessing (edge partitioning by destination, sorting,
padding to 128-edge batches, int16 index packing for dma_gather) happens on
the host in numpy inside kernel().
"""

import math
import os
import sys
from contextlib import ExitStack
from dataclasses import dataclass, field

import numpy as np

_TRN_REPO = "/opt/trn_rl_repo"
if _TRN_REPO not in sys.path and not any("trn_rl_repo" in p for p in sys.path):
    sys.path.insert(0, _TRN_REPO)

import ml_dtypes  # noqa: E402

import concourse.bass as bass  # noqa: E402
import concourse.bacc as bacc  # noqa: E402
import concourse.mybir as mybir  # noqa: E402
import concourse.tile as tile  # noqa: E402
from concourse.bass import ts  # noqa: E402
from concourse.bass_utils import run_bass_kernel_spmd  # noqa: E402

F32 = mybir.dt.float32
BF16 = mybir.dt.bfloat16
I16 = mybir.dt.int16
AF = mybir.ActivationFunctionType
ALU = mybir.AluOpType
AX = mybir.AxisListType

LN_EPS = 1e-5
P = 128  # partitions / tile rows
DMA_SCRATCH = 16384  # SWDGE descriptor carveout bytes/partition


@dataclass
class Cfg:
    n_nodes: int = 50000
    d: int = 256
    n_cores: int = 8
    n_step: int = 3
    half: int = 32768  # int16 index range for dma_gather
    # gather group sizes (dest tiles per dma_gather call) per step
    group_sizes: tuple = (3, 2, 2)
    gelu: str = "erf"  # "erf" (exact, HW), "tanh" (sim fallback)
    n_queues: int = 4  # SWDGE descriptor queues for dma_gather

    @property
    def rpc(self):  # rows per core
        return (self.n_nodes + self.n_cores - 1) // self.n_cores

    @property
    def tpc(self):  # 128-row tiles per core
        return (self.rpc + P - 1) // P

    @property
    def lp(self):  # padded local rows
        return self.tpc * P

    @property
    def ntot(self):  # padded total rows (all-gathered table size)
        return self.lp * self.n_cores


# ---------------------------------------------------------------------------
# host-side preprocessing
# ---------------------------------------------------------------------------


@dataclass
class SpmmPlan:
    step: int
    src: int  # 0 = h, 1 = state1, 2 = state2
    # per dest tile: (Blo, Bhi) batch counts (identical across cores)
    B: list = field(default_factory=list)
    TB: int = 0  # total batches = sum(Blo+Bhi)
    idx_cols: int = 0
    # per group: list of ((c0_lo, GBlo), (c0_hi, GBhi)) idx column starts
    calls: list = field(default_factory=list)
    # per tile: (bb0_lo, goff_lo, bb0_hi, goff_hi); goff = batch offset inside
    # the (group, half) gathered buffer
    tinfo: list = field(default_factory=list)


@dataclass
class Plan:
    cfg: Cfg
    spmms: list  # list[SpmmPlan]
    groups: list  # per step: list of list of dest-tile indices
    steps: list  # per step: list of spmm indices


def _pack_positions(g, cfg):
    """global node id -> (half, row) in the split half-tables.

    Each core's rows are split at hsplit = lp/2; half-0 rows of all cores
    form the "lo" table (AllGather #1), half-1 rows the "hi" table
    (AllGather #2). Both tables have n_cores*hsplit rows < 32768, so the
    int16 dma_gather indices cover them."""
    hs = cfg.lp // 2
    m = g // cfg.rpc
    r = g - m * cfg.rpc
    half = (r >= hs).astype(np.int64)
    return half, m * hs + (r - half * hs)


def make_plan_and_inputs(inputs, cfg: Cfg):
    x = np.asarray(inputs["x"], dtype=np.float32)
    adj_rows = np.asarray(inputs["adj_rows"])
    adj_cols = np.asarray(inputs["adj_cols"])
    adj_vals = np.asarray(inputs["adj_vals"], dtype=np.float32)
    idxes_seq = np.asarray(inputs["idxes_seq"]).astype(np.int64)
    idxes_res = np.asarray(inputs["idxes_res"]).astype(np.int64)
    W = np.asarray(inputs["W"], dtype=np.float32)
    b = np.asarray(inputs["b"], dtype=np.float32)
    gamma = np.asarray(inputs["gamma"], dtype=np.float32)
    beta = np.asarray(inputs["beta"], dtype=np.float32)

    nc_, d, tpc = cfg.n_cores, cfg.d, cfg.tpc

    # spmm list: (step, adj_idx, src_state)
    spmm_defs = []
    off = 0
    for i in range(cfg.n_step):
        spmm_defs.append((i, int(idxes_seq[i]), i))
        for j in range(i):
            spmm_defs.append((i, int(idxes_res[off + j]), j))
        off += i
    # order inside a step: seq first then res (matches construction order)
    steps = [[] for _ in range(cfg.n_step)]
    for k, (s, _, _) in enumerate(spmm_defs):
        steps[s].append(k)

    groups = []
    for s in range(cfg.n_step):
        gsz = cfg.group_sizes[s]
        groups.append([list(range(t0, min(t0 + gsz, tpc)))
                      for t0 in range(0, tpc, gsz)])

    # ---- bucket the edges --------------------------------------------------
    # per spmm, per core: sorted arrays + counts
    percore = []  # [k][m] -> dict(i16, rl, v, counts[t,h])
    spmms = []
    for k, (s, a, src) in enumerate(spmm_defs):
        rows = adj_rows[a].astype(np.int64)
        cols = adj_cols[a].astype(np.int64)
        vals = adj_vals[a]
        owner = rows // cfg.rpc
        half_all, ps_all = _pack_positions(cols, cfg)
        cores = []
        counts_all = np.zeros((nc_, tpc, 2), dtype=np.int64)
        for m in range(nc_):
            mask = owner == m
            lr = rows[mask] - m * cfg.rpc
            t = lr // P
            rl = (lr % P).astype(np.float32)
            h = half_all[mask]
            i16 = ps_all[mask].astype(np.int16)
            v = vals[mask]
            key = t * 2 + h
            order = np.argsort(key, kind="stable")
            key = key[order]
            cnt = np.bincount(key, minlength=tpc * 2).reshape(tpc, 2)
            counts_all[m] = cnt
            cores.append(dict(i16=i16[order], rl=rl[order], v=v[order],
                              key=key))
        cmax = counts_all.max(axis=0)  # [tpc, 2]
        B = []
        for t in range(tpc):
            blo = max(1, math.ceil(cmax[t, 0] / P))
            bhi = math.ceil(cmax[t, 1] / P)
            B.append((blo, bhi))
        sp = SpmmPlan(step=s, src=src, B=B)
        sp.TB = sum(bl + bh for bl, bh in B)
        # idx layout: per group: [lo buckets t-major][hi buckets t-major]
        # meta layout: per group: t-major: [lo batches][hi batches]
        calls = []
        c0 = 0
        for g_ts in groups[s]:
            entry = []
            for h in (0, 1):
                GB = sum(B[t][h] for t in g_ts)
                entry.append((c0, GB))
                c0 += GB * 8
            calls.append(entry)
        sp.calls = calls
        sp.idx_cols = c0
        tinfo = [None] * tpc
        bb = 0
        for g_ts in groups[s]:
            golo = 0
            gohi = 0
            # meta order within group: t asc, lo then hi
            for t in g_ts:
                bl, bh = B[t]
                tinfo[t] = (bb, golo, bb + bl, gohi)
                bb += bl + bh
                golo += bl
                gohi += bh
        sp.tinfo = tinfo
        spmms.append(sp)
        percore.append(cores)

    plan = Plan(cfg=cfg, spmms=spmms, groups=groups, steps=steps)
    plan.maxnb = max(bl + bh for sp in spmms for (bl, bh) in sp.B)

    # ---- per-core input arrays --------------------------------------------
    iota = np.broadcast_to(
        np.tile(np.arange(P, dtype=np.float32), plan.maxnb).astype(
            ml_dtypes.bfloat16), (P, plan.maxnb * P)).copy()

    in_maps = []
    for m in range(nc_):
        im = {}
        # xT shard: [d, lp] bf16 (local rows, natural order)
        r0 = m * cfg.rpc
        r1 = min((m + 1) * cfg.rpc, cfg.n_nodes)
        xs = np.zeros((cfg.lp, d), dtype=np.float32)
        xs[: r1 - r0] = x[r0:r1]
        im["xT"] = np.ascontiguousarray(xs.T).astype(ml_dtypes.bfloat16)
        im["WT"] = np.ascontiguousarray(W.T).astype(ml_dtypes.bfloat16)
        im["bias_bc"] = np.broadcast_to(b, (P, d)).copy()
        im["bias_row"] = b.reshape(1, d).astype(ml_dtypes.bfloat16)
        im["gamma_bc"] = np.broadcast_to(gamma, (P, d)).copy()
        im["beta_bc"] = np.broadcast_to(beta, (P, d)).copy()
        im["iota"] = iota

        for k, sp in enumerate(spmms):
            cd = percore[k][m]
            bounds = np.searchsorted(cd["key"], np.arange(tpc * 2 + 1))
            # --- idx array (call order: group -> half -> t) ---
            idx_chunks = []
            for gi, g_ts in enumerate(plan.groups[sp.step]):
                for h in (0, 1):
                    for t in g_ts:
                        Bn = sp.B[t][h]
                        if Bn == 0:
                            continue
                        lo_, hi_ = bounds[t * 2 + h], bounds[t * 2 + h + 1]
                        seg = cd["i16"][lo_:hi_]
                        padv = seg[-1] if len(seg) else np.int16(0)
                        pad = np.full(Bn * P - len(seg), padv, dtype=np.int16)
                        idx_chunks.append(np.concatenate([seg, pad]))
            flat = np.concatenate(idx_chunks) if idx_chunks else np.zeros(
                0, np.int16)
            cols = flat.reshape(-1, 16).T  # [16, cols]
            im[f"idx{k}"] = np.tile(cols, (8, 1)).copy()
            # --- meta arrays (order: group -> t -> lo,hi) ---
            rl_chunks = []
            v_chunks = []
            for gi, g_ts in enumerate(plan.groups[sp.step]):
                for t in g_ts:
                    for h in (0, 1):
                        Bn = sp.B[t][h]
                        if Bn == 0:
                            continue
                        lo_, hi_ = bounds[t * 2 + h], bounds[t * 2 + h + 1]
                        npad = Bn * P - (hi_ - lo_)
                        rl_chunks.append(np.concatenate(
                            [cd["rl"][lo_:hi_],
                             np.zeros(npad, np.float32)]))
                        v_chunks.append(np.concatenate(
                            [cd["v"][lo_:hi_], np.zeros(npad, np.float32)]))
            rl_flat = np.concatenate(rl_chunks)
            v_flat = np.concatenate(v_chunks)
            im[f"rloc{k}"] = np.ascontiguousarray(
                rl_flat.reshape(sp.TB, P).T).astype(ml_dtypes.bfloat16)
            im[f"vals{k}"] = np.ascontiguousarray(
                v_flat.reshape(sp.TB, P).T).astype(ml_dtypes.bfloat16)
        in_maps.append(im)

    return plan, in_maps


# ---------------------------------------------------------------------------
# device program
# ---------------------------------------------------------------------------


def _patch_lane_by_queue(n_queues):
    """Pin Tile's DMASW completion-sem lanes to SWDGE queues.

    The ucode locks each completion semaphore to one SWDGE queue; Tile's
    default round-robin over all 8 lanes ignores queue_num and mixes them.
    Give each queue a dedicated block of lanes instead."""
    from concourse import tile_sem_assignment as tsa
    if getattr(tsa.TileClockTick, "_gnn_patched", 0) == n_queues:
        return
    orig = getattr(tsa.TileClockTick, "_gnn_orig_assign_tick",
                   tsa.TileClockTick._assign_tick)

    def patched(self, inst):
        qn = getattr(inst, "queue_num", None)
        if (qn is not None and inst.engine == mybir.EngineType.Pool
                and isinstance(inst, tsa.DMAInst)):
            if not hasattr(self, "_gnn_q_rr"):
                self._gnn_q_rr = {}
            lpq = max(1, self.swdge_sem_count // n_queues)
            r = self._gnn_q_rr.get(qn, 0)
            self._gnn_q_rr[qn] = (r + 1) % lpq
            self.next_sw_dma_idx = (qn * lpq + r) % self.swdge_sem_count
        return orig(self, inst)

    tsa.TileClockTick._gnn_orig_assign_tick = orig
    tsa.TileClockTick._assign_tick = patched
    tsa.TileClockTick._gnn_patched = n_queues


def build_program(plan: Plan):
    cfg = plan.cfg
    _patch_lane_by_queue(cfg.n_queues)
    d, tpc, lp, ntot = cfg.d, cfg.tpc, cfg.lp, cfg.ntot
    nc = bacc.Bacc("TRN2", target_bir_lowering=False, debug=False,
                   num_devices=cfg.n_cores,
                   dynamic_dma_scratch_size=DMA_SCRATCH,
                   num_swdge_queues=cfg.n_queues)

    hs = lp // 2
    nt2 = hs * cfg.n_cores
    xT = nc.dram_tensor("xT", [d, lp], BF16, kind="ExternalInput")
    WT = nc.dram_tensor("WT", [d, d], BF16, kind="ExternalInput")
    bias_bc = nc.dram_tensor("bias_bc", [P, d], F32, kind="ExternalInput")
    bias_row = nc.dram_tensor("bias_row", [1, d], BF16, kind="ExternalInput")
    gamma_bc = nc.dram_tensor("gamma_bc", [P, d], F32, kind="ExternalInput")
    beta_bc = nc.dram_tensor("beta_bc", [P, d], F32, kind="ExternalInput")
    iota_d = nc.dram_tensor("iota", [P, plan.maxnb * P], BF16,
                            kind="ExternalInput")
    idx_d, rloc_d, vals_d = [], [], []
    for k, sp in enumerate(plan.spmms):
        idx_d.append(nc.dram_tensor(f"idx{k}", [P, sp.idx_cols], I16,
                                    kind="ExternalInput"))
        rloc_d.append(nc.dram_tensor(f"rloc{k}", [P, sp.TB], BF16,
                                     kind="ExternalInput"))
        vals_d.append(nc.dram_tensor(f"vals{k}", [P, sp.TB], BF16,
                                     kind="ExternalInput"))
    out_d = nc.dram_tensor("out", [lp, d], F32, kind="ExternalOutput")

    # every state is computed as a shard then all-gathered into split
    # half-tables via two pipelined AllGathers
    shards = [nc.dram_tensor(f"s{j}_shard", [lp, d], BF16)
              for j in range(cfg.n_step)]
    tabs = [(nc.dram_tensor(f"s{j}_lo", [nt2, d], BF16,
                            addr_space="Shared"),
             nc.dram_tensor(f"s{j}_hi", [nt2, d], BF16,
                            addr_space="Shared"))
            for j in range(cfg.n_step)]
    RG = [list(range(cfg.n_cores))]

    with ExitStack() as ctx:
        tc = ctx.enter_context(tile.TileContext(nc, num_cores=cfg.n_cores))
        const = ctx.enter_context(tc.tile_pool(name="const", bufs=1))

        iota_sb = const.tile([P, plan.maxnb * P], BF16)
        nc.sync.dma_start(iota_sb[:], iota_d[:, :])
        bias_sb = const.tile([P, d], F32)
        nc.sync.dma_start(bias_sb[:], bias_bc[:, :])
        gamma_sb = const.tile([P, d], F32)
        nc.sync.dma_start(gamma_sb[:], gamma_bc[:, :])
        beta_sb = const.tile([P, d], F32)
        nc.sync.dma_start(beta_sb[:], beta_bc[:, :])
        eps_sb = const.tile([P, 1], F32)
        nc.vector.memset(eps_sb[:], LN_EPS)
        half_sb = const.tile([P, 1], F32)
        nc.vector.memset(half_sb[:], 0.5)

        # ---------------- phase: h = x @ W.T + b (shard + split AGs) ------
        # bias folded into the PSUM accumulation via a K=1 ones x b matmul.
        ones_sb = const.tile([1, P], BF16)
        nc.vector.memset(ones_sb[:], 1.0)
        brow_sb = const.tile([1, d], BF16)
        nc.sync.dma_start(brow_sb[:], bias_row[:, :])
        with tc.tile_pool(name="hph", bufs=1) as hp, \
                tc.tile_pool(name="hpsum", bufs=8, space="PSUM") as psh, \
                tc.tile_pool(name="hout", bufs=6) as hop:
            wt0 = hp.tile([P, d], BF16, tag="wt0")
            nc.sync.dma_start(wt0[:], WT[0:P, :])
            wt1 = hp.tile([P, d], BF16, tag="wt1")
            nc.sync.dma_start(wt1[:], WT[P:2 * P, :])
            xt0 = hp.tile([P, lp], BF16, tag="xt0")
            nc.sync.dma_start(xt0[:], xT[0:P, :])
            xt1 = hp.tile([P, lp], BF16, tag="xt1")
            nc.sync.dma_start(xt1[:], xT[P:2 * P, :])
            for t in range(tpc):
                psum = psh.tile([P, d], F32)
                nc.tensor.matmul(psum[:], xt0[:, ts(t, P)], wt0[:],
                                 start=True, stop=False)
                nc.tensor.matmul(psum[:], xt1[:, ts(t, P)], wt1[:],
                                 start=False, stop=False)
                nc.tensor.matmul(psum[:], ones_sb[:], brow_sb[:],
                                 start=False, stop=True)
                hsb = hop.tile([P, d], BF16)
                nc.vector.tensor_copy(hsb[:], psum[:])
                nc.sync.dma_start(shards[0][ts(t, P), :], hsb[:])
        nc.gpsimd.collective_compute(
            "AllGather", ALU.bypass, replica_groups=RG,
            ins=[shards[0][0:hs, :]], outs=[tabs[0][0][:, :]])
        nc.gpsimd.collective_compute(
            "AllGather", ALU.bypass, replica_groups=RG,
            ins=[shards[0][hs:lp, :]], outs=[tabs[0][1][:, :]])

        # ---------------- spmm steps --------------------------------------
        for s in range(cfg.n_step):
            contribs = plan.steps[s]
            maxgb = [[1, 1] for _ in contribs]
            for ci, k in enumerate(contribs):
                for entry in plan.spmms[k].calls:
                    for h in (0, 1):
                        maxgb[ci][h] = max(maxgb[ci][h], entry[h][1])
            with ExitStack() as sctx:
                mp = sctx.enter_context(
                    tc.tile_pool(name=f"meta{s}", bufs=1))
                ip = sctx.enter_context(
                    tc.tile_pool(name=f"idxp{s}", bufs=4))
                gp = sctx.enter_context(
                    tc.tile_pool(name=f"gath{s}",
                                 bufs=(4, 3, 2)[min(s, 2)]))
                vp = sctx.enter_context(
                    tc.tile_pool(name=f"vh{s}", bufs=3))
                pp = sctx.enter_context(
                    tc.tile_pool(name=f"ps{s}", bufs=8, space="PSUM"))
                op = sctx.enter_context(
                    tc.tile_pool(name=f"so{s}", bufs=3))

                rloc_sb, vals_sb = {}, {}
                maxixg = {}
                for k in contribs:
                    sp = plan.spmms[k]
                    rloc_sb[k] = mp.tile([P, sp.TB], BF16, tag=f"rl{k}",
                                         name=f"rl{k}")
                    nc.sync.dma_start(rloc_sb[k][:], rloc_d[k][:, :])
                    vals_sb[k] = mp.tile([P, sp.TB], BF16, tag=f"vl{k}",
                                         name=f"vl{k}")
                    nc.sync.dma_start(vals_sb[k][:], vals_d[k][:, :])
                    maxixg[k] = max((c[0][1] + c[1][1]) * 8
                                    for c in sp.calls)

                nreg = nc.gpsimd.alloc_register(f"nidx{s}")
                qctr = 0
                for gi, g_ts in enumerate(plan.groups[s]):
                    gt = {}
                    for ci, k in enumerate(contribs):
                        sp = plan.spmms[k]
                        (c0_lo, GBlo), (c0_hi, GBhi) = sp.calls[gi]
                        cols_g = (GBlo + GBhi) * 8
                        ixt = ip.tile([P, maxixg[k]], I16, tag=f"ixg{k}",
                                      name=f"ixg{k}")
                        nc.sync.dma_start(ixt[:, 0:cols_g],
                                          idx_d[k][:, c0_lo:c0_lo + cols_g])
                        for h, GB, cg0 in ((0, GBlo, 0), (1, GBhi, GBlo * 8)):
                            if GB == 0:
                                continue
                            g_tile = gp.tile(
                                [P, maxgb[ci][h], d], BF16,
                                tag=f"g{k}_{h}",
                                bufs=(3 if s == cfg.n_step - 1
                                      and plan.spmms[k].src < s else None))
                            in_ap = tabs[sp.src][h][:, :]
                            nc.gpsimd.reg_mov(nreg, GB * P)
                            nc.gpsimd.dma_gather(
                                g_tile[:, 0:GB, :], in_ap,
                                ixt[:, cg0:cg0 + GB * 8],
                                num_idxs=GB * P, num_idxs_reg=nreg,
                                elem_size=d,
                                single_packet=(GB * P <= 1024),
                                queue_num=(ci * 2 + h) % cfg.n_queues
                                if len(contribs) > 1
                                else qctr % cfg.n_queues)
                            qctr += 1
                            gt[(k, h)] = g_tile
                    for t in g_ts:
                        nmm = sum(plan.spmms[k].B[t][0] +
                                  plan.spmms[k].B[t][1] for k in contribs)
                        psum = pp.tile([P, d], F32)
                        mi = 0
                        for k in contribs:
                            sp = plan.spmms[k]
                            bb_lo, go_lo, bb_hi, go_hi = sp.tinfo[t]
                            blo, bhi = sp.B[t]
                            nb = blo + bhi
                            # val-scaled one-hot for ALL nb batches of this
                            # (spmm, tile) in two broadcast DVE ops
                            vh = vp.tile([P, nb * P], BF16)
                            vh3 = vh[:].rearrange("p (b f) -> p b f", f=P)
                            io3 = iota_sb[:, 0:nb * P].rearrange(
                                "p (b f) -> p b f", f=P)
                            nc.vector.tensor_tensor(
                                vh3, io3,
                                rloc_sb[k][:, bb_lo:bb_lo + nb].to_broadcast(
                                    (P, nb, P)),
                                op=ALU.is_equal)
                            nc.vector.tensor_tensor(
                                vh3, vh3,
                                vals_sb[k][:, bb_lo:bb_lo + nb].to_broadcast(
                                    (P, nb, P)),
                                op=ALU.mult)
                            for h, nbh, go0, boff in ((0, blo, go_lo, 0),
                                                      (1, bhi, go_hi, blo)):
                                for bi in range(nbh):
                                    nc.tensor.matmul(
                                        psum[:], vh3[:, boff + bi, :],
                                        gt[(k, h)][:, go0 + bi, :],
                                        start=(mi == 0),
                                        stop=(mi == nmm - 1))
                                    mi += 1
                        if s < cfg.n_step - 1:
                            osb = op.tile([P, d], BF16, tag="osb")
                            nc.vector.tensor_copy(osb[:], psum[:])
                            nc.sync.dma_start(
                                shards[s + 1][ts(t, P), :], osb[:])
                        else:
                            _ln_gelu(nc, op, psum, gamma_sb, beta_sb,
                                     eps_sb, half_sb, out_d, t, cfg)
            if s < cfg.n_step - 1:
                nc.gpsimd.collective_compute(
                    "AllGather", ALU.bypass, replica_groups=RG,
                    ins=[shards[s + 1][0:hs, :]],
                    outs=[tabs[s + 1][0][:, :]])
                nc.gpsimd.collective_compute(
                    "AllGather", ALU.bypass, replica_groups=RG,
                    ins=[shards[s + 1][hs:lp, :]],
                    outs=[tabs[s + 1][1][:, :]])

    # Bacc.compile (via finalize) legalizes multi-waits into event
    # semaphores, auto-inserts gpsimd library loads for dma_gather, and
    # populates extended-ISA instruction bytes.
    nc.finalize()
    return nc


def _ln_gelu(nc, pool, psum, gamma_sb, beta_sb, eps_sb, half_sb, out_d, t,
             cfg: Cfg):
    d = cfg.d
    y = pool.tile([P, d], F32, tag="ln_y")
    nc.vector.tensor_copy(y[:], psum[:])
    negmu = pool.tile([P, 1], F32, tag="ln_mu")
    nc.vector.tensor_reduce(negmu[:], y[:], axis=AX.X, op=ALU.add)
    nc.scalar.mul(negmu[:], negmu[:], -1.0 / d)
    nc.scalar.add(y[:], y[:], negmu[:])  # y = centered
    sq = pool.tile([P, d], F32, tag="ln_sq")
    nc.scalar.activation(sq[:], y[:], AF.Square)
    var = pool.tile([P, 1], F32, tag="ln_var")
    nc.vector.tensor_reduce(var[:], sq[:], axis=AX.X, op=ALU.add)
    istd = pool.tile([P, 1], F32, tag="ln_istd")
    nc.scalar.activation(istd[:], var[:], AF.Sqrt, bias=eps_sb[:],
                         scale=1.0 / d)
    nc.vector.reciprocal(out=istd[:], in_=istd[:])
    nc.scalar.mul(y[:], y[:], istd[:])  # ACT: per-partition scale
    nc.vector.tensor_mul(y[:], y[:], gamma_sb[:])
    nc.vector.tensor_add(y[:], y[:], beta_sb[:])  # y = ln output
    er = pool.tile([P, d], F32, tag="ln_er")
    if cfg.gelu == "erf":
        nc.scalar.activation(er[:], y[:], AF.Erf,
                             scale=float(1.0 / np.sqrt(2.0)))
    else:  # tanh approx (CoreSim has no Erf/Gelu)
        nc.scalar.activation(sq[:], y[:], AF.Square)
        nc.vector.tensor_scalar(sq[:], sq[:], 0.044715, 1.0,
                                op0=ALU.mult, op1=ALU.add)
        nc.vector.tensor_mul(sq[:], sq[:], y[:])
        nc.scalar.activation(er[:], sq[:], AF.Tanh,
                             scale=float(np.sqrt(2.0 / np.pi)))
    # (er + 1) * 0.5 on ACT: 0.5*er + 0.5
    nc.scalar.activation(er[:], er[:], AF.Identity, bias=half_sb[:],
                         scale=0.5)
    nc.vector.tensor_mul(er[:], er[:], y[:])
    nc.sync.dma_start(out_d[ts(t, P), :], er[:])


# ---------------------------------------------------------------------------
# entry point
# ---------------------------------------------------------------------------


def run_on_hw(plan, in_maps, trace=False, **kw):
    nc = build_program(plan)
    cfg = plan.cfg
    res = run_bass_kernel_spmd(
        nc, in_maps, core_ids=list(range(cfg.n_cores)), trace=trace, **kw)
    outs = [res.results[m]["out"] for m in range(cfg.n_cores)]
    full = np.concatenate([o[: cfg.rpc] for o in outs], axis=0)[: cfg.n_nodes]
    return np.ascontiguousarray(full.astype(np.float32)), res


def kernel(**inputs):
    cfg = Cfg()
    plan, in_maps = make_plan_and_inputs(inputs, cfg)
    out, _ = run_on_hw(plan, in_maps)
    return out
